# revision 1
# baseline (speedup 1.0000x reference)
"""3-layer GAT (DGL GATConv semantics) on 8 Trainium2 NeuronCores.

Strategy (graph-parallel, per sharding hint):
  - Host load-balances dst nodes into 8*49 windows of 128 dsts (LPT packing)
    so every core/window has near-equal edge counts; node order is permuted
    accordingly and the output inverse-permuted at the end.
  - Per layer: each core projects its own nodes ([feat|el|er] in one matmul,
    since el = (h@W)@al_diag = h@(W@al_diag)), packs [feat_bf16|el_f32] into a
    768B G-row, AllGathers G across cores; er goes to a local 256B-padded
    per-node table.
  - Edge phase per 128-dst window: dma_gather the edges' source G-rows
    (edges pre-sorted by dst into windows; int16 gather indices handled by a
    lo/hi source-offset split), dma_gather er per edge from the padded er
    table (indices = dst rows), compute ex=exp(LeakyReLU(el+er)) batched,
    build the edge->dst one-hot with one batched iota-compare, weight gathered
    feats by ex in one batched multiply, then one PE matmul per 128-edge tile
    accumulates the unnormalized aggregation and one more the softmax
    denominator into PSUM (normalization divides at the end; segment-max is
    unnecessary because logits are small and exp cannot overflow fp32).
"""

from contextlib import ExitStack

import numpy as np

import concourse.bass as bass  # noqa: F401
import concourse.bacc as bacc
import concourse.mybir as mybir
import concourse.tile as tile
from concourse import bass_utils

F32 = mybir.dt.float32
BF16 = mybir.dt.bfloat16
I16 = mybir.dt.int16

GROW = 384  # bf16 elements per G row: [feat 256 | el-as-f32 8 | pad] = 768B
ERW = 64    # f32 elements per padded er row (256B)
GMAX = 1024  # max indices per dma_gather call with single_packet=True


class Cfg:
    def __init__(self, n, e, fin, h, dh, ncores, wpc, lo_rows=32768):
        self.N, self.E, self.FIN, self.H, self.DH = n, e, fin, h, dh
        self.HID = h * dh
        self.NCORES, self.WPC = ncores, wpc
        self.NPC = wpc * 128            # padded nodes per core
        self.NPAD = ncores * self.NPC   # padded global nodes
        assert self.NPAD >= n
        self.LO_ROWS = min(lo_rows, self.NPAD)
        self.HI_OFF = max(self.NPAD - self.LO_ROWS, 0)
        self.KT = self.HID // 128       # K tiles for layers 1,2
        self.KT0 = fin // 128           # K tiles for layer 0


def _lpt_windows(deg, cfg):
    """Assign nodes to ncores*wpc windows of exactly 128 slots, balancing
    per-window edge counts (LPT greedy). Returns perm_pos[node] -> slot."""
    import heapq

    nw = cfg.NCORES * cfg.WPC
    order = np.argsort(-deg, kind="stable")
    heap = [(0, w) for w in range(nw)]
    heapq.heapify(heap)
    counts = np.zeros(nw, np.int64)
    wsum = np.zeros(nw, np.int64)
    assign = np.empty(cfg.N, np.int64)
    for n in order:
        while True:
            s, w = heapq.heappop(heap)
            if counts[w] < 128:
                break
        assign[n] = w
        counts[w] += 1
        wsum[w] += deg[n]
        if counts[w] < 128:
            heapq.heappush(heap, (wsum[w], w))
    perm_pos = np.empty(cfg.N, np.int64)
    next_row = np.zeros(nw, np.int64)
    for n in range(cfg.N):
        w = assign[n]
        perm_pos[n] = w * 128 + next_row[w]
        next_row[w] += 1
    return perm_pos


def _wrap16(vals, cap):
    """Wrap a list of idx values into the Q7 [16, cap//16] layout."""
    out = np.zeros((16, cap // 16), np.int16)
    j = np.arange(len(vals))
    out[j % 16, j // 16] = vals.astype(np.int16)
    return out


def preprocess(x, edge_index, cfg):
    src = np.asarray(edge_index[0], dtype=np.int64)
    dst = np.asarray(edge_index[1], dtype=np.int64)
    deg = np.bincount(dst, minlength=cfg.N)
    perm_pos = _lpt_windows(deg, cfg)

    psrc = perm_pos[src]
    pdst = perm_pos[dst]
    gw = pdst // 128
    row = pdst % 128

    ncores, wpc = cfg.NCORES, cfg.WPC
    core = gw // wpc
    wi = gw % wpc
    is_lo = psrc < cfg.LO_ROWS

    ek = (core * wpc + wi) * 2 + (~is_lo).astype(np.int64)
    sort_idx = np.lexsort((row, ek))
    s_psrc = psrc[sort_idx]
    s_row = row[sort_idx]
    s_ek = ek[sort_idx]
    grp_start = np.searchsorted(s_ek, np.arange(ncores * wpc * 2 + 1))

    lo_t = np.zeros(wpc, np.int64)
    hi_t = np.zeros(wpc, np.int64)
    for w in range(wpc):
        for c in range(ncores):
            g = (c * wpc + w) * 2
            nlo = grp_start[g + 1] - grp_start[g]
            nhi = grp_start[g + 2] - grp_start[g + 1]
            lo_t[w] = max(lo_t[w], (nlo + 127) // 128)
            hi_t[w] = max(hi_t[w], (nhi + 127) // 128)
    T = lo_t + hi_t
    TT = int(T.sum())
    toff = np.concatenate([[0], np.cumsum(T)]).astype(np.int64)
    nidx = TT * 8

    idx16 = np.zeros((ncores, 16, nidx), np.int16)
    dstrow = np.full((ncores, 128, TT), -1, np.float32)
    sten = np.zeros((ncores, 128, TT, 2), np.float32)

    for c in range(ncores):
        for w in range(wpc):
            base_t = toff[w]
            for reg in (0, 1):
                g = (c * wpc + w) * 2 + reg
                lo, hi_ = grp_start[g], grp_start[g + 1]
                e_ps = s_psrc[lo:hi_]
                e_r = s_row[lo:hi_]
                nt = lo_t[w] if reg == 0 else hi_t[w]
                if nt == 0:
                    assert len(e_ps) == 0
                    continue
                cap = nt * 128
                n_e = len(e_ps)
                vals = np.zeros(cap, np.int64)
                vals[:n_e] = e_ps if reg == 0 else e_ps - cfg.HI_OFF
                assert (vals >= 0).all() and (vals < cfg.LO_ROWS).all()
                rt0 = base_t + (0 if reg == 0 else lo_t[w])
                idx16[c, :, rt0 * 8:(rt0 + nt) * 8] = _wrap16(vals, cap)
                rows = np.full(cap, -1, np.int64)
                rows[:n_e] = e_r
                dstrow[c, :, rt0:rt0 + nt] = rows.reshape(nt, 128).T
                seg = np.searchsorted(e_r, np.arange(129))
                for tl in range(nt):
                    b = tl * 128
                    sten[c, :, rt0 + tl, 0] = np.clip(seg[:-1] - b, 0, 128)
                    sten[c, :, rt0 + tl, 1] = np.clip(seg[1:] - b, 0, 128)

    xT = np.zeros((ncores, cfg.FIN, cfg.NPC), np.float32)
    inv_rows = np.full(cfg.NPAD, -1, np.int64)
    inv_rows[perm_pos] = np.arange(cfg.N)
    xf = np.asarray(x, np.float32)
    for c in range(ncores):
        sl = inv_rows[c * cfg.NPC:(c + 1) * cfg.NPC]
        valid = sl >= 0
        xc = np.zeros((cfg.NPC, cfg.FIN), np.float32)
        xc[valid] = xf[sl[valid]]
        xT[c] = xc.T

    # HW Q7 cores each read their own 16-partition group: replicate.
    idx16 = np.tile(idx16, (1, 8, 1))

    return dict(perm_pos=perm_pos, lo_t=lo_t, hi_t=hi_t, T=T, TT=TT,
                toff=toff, nidx=nidx, idx16=idx16,
                dstrow=dstrow, sten=sten, xT=xT)


def pack_weights(cfg, Ws, als, ars, bs, resW0):
    kts = []
    for l, W in enumerate(Ws):
        ALf = np.zeros((cfg.HID, cfg.H), np.float32)
        ARf = np.zeros((cfg.HID, cfg.H), np.float32)
        for h in range(cfg.H):
            ALf[h * cfg.DH:(h + 1) * cfg.DH, h] = als[l][h]
            ARf[h * cfg.DH:(h + 1) * cfg.DH, h] = ars[l][h]
        Wc = np.concatenate([W, W @ ALf, W @ ARf], axis=1)  # [fin, 264]
        fin = W.shape[0]
        for k in range(fin // 128):
            kts.append(Wc[k * 128:(k + 1) * 128])
    w_all = np.transpose(np.stack(kts), (1, 0, 2)).copy()  # [128, nk, 264]
    b_rep = np.stack([np.tile(b[None, :], (128, 1)) for b in bs], axis=1)
    return w_all.astype(np.float32), b_rep.astype(np.float32), \
        resW0.astype(np.float32)


def _gather(nc, out_ap, in_ap, idx_sb, col0, n, elem):
    """One dma_gather call for n indices (single_packet only up to the
    64-desc/engine packet ceiling)."""
    nc.gpsimd.dma_gather(
        out_ap=out_ap, in_ap=in_ap,
        idxs_ap=idx_sb[:, col0: col0 + n // 16],
        num_idxs=n, num_idxs_reg=n, elem_size=elem,
        single_packet=(n <= GMAX))


def build_program(cfg, meta, num_cores):
    nc = bacc.Bacc("TRN2", target_bir_lowering=False, debug=False,
                   num_devices=num_cores)
    NPC, HID = cfg.NPC, cfg.HID
    WPC = cfg.WPC
    NK = cfg.KT0 + 2 * cfg.KT
    lo_t, hi_t, T, toff, TT = meta["lo_t"], meta["hi_t"], meta["T"], \
        meta["toff"], meta["TT"]

    d_xT = nc.dram_tensor("xT", [cfg.FIN, NPC], F32, kind="ExternalInput")
    d_w = nc.dram_tensor("w_all", [128, NK, 264], F32, kind="ExternalInput")
    d_rw = nc.dram_tensor("resW0", [128, 256], F32, kind="ExternalInput")
    d_b = nc.dram_tensor("b_rep", [128, 3, 256], F32, kind="ExternalInput")
    d_id = nc.dram_tensor("ident", [128, 128], F32, kind="ExternalInput")
    d_idx = nc.dram_tensor("idx16", [128, meta["nidx"]], I16,
                           kind="ExternalInput")
    d_dr = nc.dram_tensor("dstrow", [128, TT], F32, kind="ExternalInput")
    d_st = nc.dram_tensor("sten", [128, TT, 2], F32, kind="ExternalInput")
    d_out = nc.dram_tensor("out", [NPC, cfg.DH], F32, kind="ExternalOutput")

    maxT = int(T.max())

    with ExitStack() as ctx:
        tc = ctx.enter_context(tile.TileContext(nc))
        cpool = ctx.enter_context(tc.tile_pool(name="const", bufs=1))
        dram = ctx.enter_context(tc.tile_pool(name="dram", bufs=1,
                                              space="DRAM"))
        fgpool = ctx.enter_context(tc.tile_pool(name="fg", bufs=2))
        ohpool = ctx.enter_context(tc.tile_pool(name="oh", bufs=2))
        epool = ctx.enter_context(tc.tile_pool(name="e", bufs=3))
        wpool = ctx.enter_context(tc.tile_pool(name="wt", bufs=2))
        hpool = ctx.enter_context(tc.tile_pool(name="h", bufs=4))
        gpool = ctx.enter_context(tc.tile_pool(name="g", bufs=3))
        ps_m = ctx.enter_context(tc.tile_pool(name="psm", bufs=2,
                                              space="PSUM"))
        ps_p = ctx.enter_context(tc.tile_pool(name="psp", bufs=3,
                                              space="PSUM"))
        ps_e = ctx.enter_context(tc.tile_pool(name="pse", bufs=2,
                                              space="PSUM"))

        g_loc = dram.tile([NPC, GROW], BF16)
        g_fulls = [
            dram.tile([cfg.NPAD, GROW], BF16, name=f"g_full{i}",
                      addr_space="Shared" if num_cores > 4 else "Local")
            for i in range(3)]
        hbuf = [dram.tile([NPC, HID], F32, name="hbuf0"),
                dram.tile([NPC, HID], F32, name="hbuf1")]
        res0 = dram.tile([NPC, HID], F32)

        w_sb = cpool.tile([128, NK, 264], F32)
        nc.sync.dma_start(w_sb[:], d_w[:])
        rw_sb = cpool.tile([128, 256], F32)
        nc.sync.dma_start(rw_sb[:], d_rw[:])
        b_sb = cpool.tile([128, 3, 256], F32)
        nc.sync.dma_start(b_sb[:], d_b[:])
        id_sb = cpool.tile([128, 128], F32)
        nc.sync.dma_start(id_sb[:], d_id[:])
        idx_sb = cpool.tile([128, meta["nidx"]], I16)
        nc.sync.dma_start(idx_sb[:], d_idx[:])
        dr_sb = cpool.tile([128, TT], F32)
        nc.sync.dma_start(dr_sb[:], d_dr[:])
        st_sb = cpool.tile([128, TT, 2], F32)
        nc.sync.dma_start(st_sb[:], d_st[:])
        er_res = cpool.tile([128, WPC, 4], BF16)
        iota_sb = cpool.tile([128, 128], F32)
        nc.gpsimd.iota(iota_sb[:], pattern=[[1, 128]], base=0,
                       channel_multiplier=0,
                       allow_small_or_imprecise_dtypes=True)

        kt_of_layer = [list(range(cfg.KT0)),
                       list(range(cfg.KT0, cfg.KT0 + cfg.KT)),
                       list(range(cfg.KT0 + cfg.KT, NK))]

        for l in range(3):
            # ---------------- projection phase ----------------
            for nt in range(WPC):
                kts = kt_of_layer[l]
                lhsTs = []
                if l == 0:
                    xt = hpool.tile([128, 128], F32, tag="lhsT")
                    nc.sync.dma_start(xt[:], d_xT[:, nt * 128:(nt + 1) * 128])
                    lhsTs.append(xt)
                else:
                    h_in = hpool.tile([128, HID], F32, tag="hin")
                    nc.sync.dma_start(
                        h_in[:], hbuf[(l + 1) % 2][nt * 128:(nt + 1) * 128, :])
                    for ft in range(cfg.KT):
                        pst = ps_p.tile([128, 128], F32, tag="pp")
                        nc.tensor.transpose(
                            pst[:], h_in[:, ft * 128:(ft + 1) * 128], id_sb[:])
                        hT = hpool.tile([128, 128], F32, tag="lhsT")
                        nc.scalar.copy(hT[:], pst[:])
                        lhsTs.append(hT)
                pp = ps_p.tile([128, 264], F32, tag="pp")
                for k, (kt, lt) in enumerate(zip(kts, lhsTs)):
                    nc.tensor.matmul(pp[:], lt[:], w_sb[:, kt, :],
                                     start=(k == 0), stop=(k == len(kts) - 1))
                g_sb = gpool.tile([128, 264], BF16)
                nc.scalar.copy(g_sb[:, 0:256], pp[:, 0:256])
                nc.scalar.copy(g_sb[:, 256:264].bitcast(F32), pp[:, 256:260])
                nc.scalar.copy(er_res[:, nt, :], pp[:, 260:264])
                nc.sync.dma_start(g_loc[nt * 128:(nt + 1) * 128, 0:264],
                                  g_sb[:, 0:264])
                if l == 0:
                    pr = ps_p.tile([128, 256], F32, tag="pp")
                    nc.tensor.matmul(pr[:], lhsTs[0][:], rw_sb[:],
                                     start=True, stop=True)
                    r_sb = gpool.tile([128, 256], F32, tag="res")
                    nc.scalar.copy(r_sb[:], pr[:])
                    nc.sync.dma_start(res0[nt * 128:(nt + 1) * 128, :],
                                      r_sb[:])

            g_full = g_fulls[l]
            nc.gpsimd.collective_compute(
                "AllGather", mybir.AluOpType.bypass,
                replica_groups=[list(range(num_cores))],
                ins=[g_loc.opt()], outs=[g_full.opt()])

            # ---------------- aggregation phase ----------------
            for w in range(WPC):
                Tw = int(T[w])
                lt_, ht_ = int(lo_t[w]), int(hi_t[w])
                t0 = int(toff[w])
                fg = fgpool.tile([128, maxT, GROW], BF16)
                if lt_ > 0:
                    _gather(nc, fg[:, 0:lt_, :], g_full[0:cfg.LO_ROWS, :],
                            idx_sb, t0 * 8, lt_ * 128, GROW)
                if ht_ > 0:
                    _gather(nc, fg[:, lt_:Tw, :],
                            g_full[cfg.HI_OFF:cfg.NPAD, :], idx_sb,
                            (t0 + lt_) * 8, ht_ * 128, GROW)

                # staircase one-hot (dsts-on-partitions), batched:
                # mst[d, t, e] = (st[d,t] <= e < en[d,t])
                mst = ohpool.tile([128, maxT, 128], BF16, tag="mst")
                iota3 = iota_sb[:].unsqueeze(1).broadcast_to([128, Tw, 128])
                nc.vector.tensor_tensor(
                    mst[:, 0:Tw, :], iota3,
                    st_sb[:, t0:t0 + Tw, 0:1].broadcast_to([128, Tw, 128]),
                    mybir.AluOpType.is_ge)
                mlt = ohpool.tile([128, maxT, 128], BF16, tag="mlt")
                nc.vector.tensor_tensor(
                    mlt[:, 0:Tw, :], iota3,
                    st_sb[:, t0:t0 + Tw, 1:2].broadcast_to([128, Tw, 128]),
                    mybir.AluOpType.is_lt)
                nc.vector.tensor_tensor(
                    mst[:, 0:Tw, :], mst[:, 0:Tw, :], mlt[:, 0:Tw, :],
                    mybir.AluOpType.mult)
                pe_er = ps_e.tile([128, maxT, 4], F32)
                for t in range(Tw):
                    nc.tensor.matmul(pe_er[:, t, :], mst[:, t, :],
                                     er_res[:, w, :], start=True, stop=True)

                # one-hot (edges-on-partitions): mt[e, t, d] = (row_e,t == d)
                mt = ohpool.tile([128, maxT, 128], BF16)
                nc.vector.tensor_tensor(
                    mt[:, 0:Tw, :],
                    dr_sb[:, t0:t0 + Tw].unsqueeze(2)
                         .broadcast_to([128, Tw, 128]),
                    iota_sb[:].unsqueeze(1).broadcast_to([128, Tw, 128]),
                    mybir.AluOpType.is_equal)

                # logits, batched over the window
                el_v = fg[:, 0:Tw, 256:264].bitcast(F32)    # [128, Tw, 4]
                e_sb = epool.tile([128, maxT, 4], F32, tag="e")
                nc.vector.tensor_tensor(e_sb[:, 0:Tw, :], el_v,
                                        pe_er[:, 0:Tw, :],
                                        mybir.AluOpType.add)
                nc.vector.scalar_tensor_tensor(
                    e_sb[:, 0:Tw, :], e_sb[:, 0:Tw, :], 0.2, e_sb[:, 0:Tw, :],
                    mybir.AluOpType.mult, mybir.AluOpType.max)
                ex_sb = epool.tile([128, maxT, 4], F32, tag="ex")
                nc.scalar.activation(ex_sb[:, 0:Tw, :], e_sb[:, 0:Tw, :],
                                     mybir.ActivationFunctionType.Exp)
                exb = epool.tile([128, maxT, 4], BF16, tag="exb")
                nc.vector.tensor_copy(exb[:, 0:Tw, :], ex_sb[:, 0:Tw, :])

                # weighted feats, batched: wsb[e,t,h,d] = fg[e,t,h,d]*ex[e,t,h]
                wsb = wpool.tile([128, maxT, 256], BF16)
                nc.vector.tensor_tensor(
                    wsb[:, 0:Tw, :].rearrange("p t (h d) -> p t h d", h=4),
                    fg[:, 0:Tw, 0:256].rearrange("p t (h d) -> p t h d", h=4),
                    exb[:, 0:Tw, :].unsqueeze(3).broadcast_to(
                        [128, Tw, 4, 64]),
                    mybir.AluOpType.mult)

                pm = ps_m.tile([128, 260], F32)
                for t in range(Tw):
                    nc.tensor.matmul(pm[:, 0:256], mt[:, t, :], wsb[:, t, :],
                                     start=(t == 0), stop=(t == Tw - 1),
                                     skip_group_check=True)
                for t in range(Tw):
                    nc.tensor.matmul(pm[:, 256:260], mt[:, t, :],
                                     exb[:, t, :],
                                     start=(t == 0), stop=(t == Tw - 1),
                                     skip_group_check=True)

                den = epool.tile([128, 4], F32, tag="den")
                nc.vector.tensor_scalar(den[:], pm[:, 256:260], 1e-16, None,
                                        mybir.AluOpType.max)
                rden = epool.tile([128, 4], F32, tag="rden")
                nc.vector.reciprocal(rden[:], den[:])
                hn = hpool.tile([128, HID], F32, tag="hn")
                nc.vector.tensor_tensor(
                    hn[:].rearrange("p (h d) -> p h d", h=4),
                    pm[:, 0:256].rearrange("p (h d) -> p h d", h=4),
                    rden[:].unsqueeze(2).broadcast_to([128, 4, 64]),
                    mybir.AluOpType.mult)
                rsb = hpool.tile([128, HID], F32, tag="res_in")
                rsrc = res0 if l == 0 else hbuf[(l + 1) % 2]
                nc.sync.dma_start(rsb[:], rsrc[w * 128:(w + 1) * 128, :])
                nc.vector.tensor_tensor(hn[:], hn[:], rsb[:],
                                        mybir.AluOpType.add)
                nc.vector.tensor_tensor(hn[:], hn[:], b_sb[:, l, :],
                                        mybir.AluOpType.add)
                if l < 2:
                    nc.scalar.activation(hn[:], hn[:],
                                         mybir.ActivationFunctionType.Relu)
                    nc.sync.dma_start(hbuf[l % 2][w * 128:(w + 1) * 128, :],
                                      hn[:])
                else:
                    osb = hpool.tile([128, cfg.DH], F32, tag="osb")
                    nc.vector.tensor_reduce(
                        osb[:],
                        hn[:].rearrange("p (h d) -> p d h", h=4),
                        mybir.AxisListType.X, mybir.AluOpType.add)
                    nc.vector.tensor_scalar(osb[:], osb[:], 1.0 / cfg.H, None,
                                            mybir.AluOpType.mult)
                    nc.sync.dma_start(d_out[w * 128:(w + 1) * 128, :], osb[:])

    nc.compile()
    return nc


def make_in_maps(cfg, meta, wnp, num_cores):
    w_all, b_rep, rw = wnp
    ident = np.eye(128, dtype=np.float32)
    maps = []
    for c in range(num_cores):
        maps.append({
            "xT": meta["xT"][c],
            "w_all": w_all, "resW0": rw, "b_rep": b_rep, "ident": ident,
            "idx16": meta["idx16"][c],
            "dstrow": meta["dstrow"][c],
            "sten": meta["sten"][c],
        })
    return maps


def assemble_output(cfg, meta, results):
    out = np.empty((cfg.N, cfg.DH), np.float32)
    full = np.concatenate([r["out"] for r in results], axis=0)
    out[:] = full[meta["perm_pos"]]
    return out


def solve(x, edge_index, Ws, als, ars, bs, resW0, cfg, trace=False):
    meta = preprocess(x, edge_index, cfg)
    wnp = pack_weights(cfg, Ws, als, ars, bs, resW0)
    nc = build_program(cfg, meta, cfg.NCORES)
    in_maps = make_in_maps(cfg, meta, wnp, cfg.NCORES)
    res = bass_utils.run_bass_kernel_spmd(
        nc, in_maps, core_ids=list(range(cfg.NCORES)), trace=trace)
    out = assemble_output(cfg, meta, res.results)
    return out, res


def kernel(x, edge_index, W0, W1, W2, al0, al1, al2, ar0, ar1, ar2,
           b0, b1, b2, resW0):
    cfg = Cfg(n=50000, e=800000, fin=128, h=4, dh=64, ncores=8, wpc=49)
    out, _ = solve(np.asarray(x, np.float32), np.asarray(edge_index),
                   [np.asarray(W0, np.float32), np.asarray(W1, np.float32),
                    np.asarray(W2, np.float32)],
                   [np.asarray(al0, np.float32), np.asarray(al1, np.float32),
                    np.asarray(al2, np.float32)],
                   [np.asarray(ar0, np.float32), np.asarray(ar1, np.float32),
                    np.asarray(ar2, np.float32)],
                   [np.asarray(b0, np.float32), np.asarray(b1, np.float32),
                    np.asarray(b2, np.float32)],
                   np.asarray(resW0, np.float32), cfg)
    return out



# revision 5
# speedup vs baseline: 1.2054x; 1.2054x over previous
"""3-layer GAT (DGL GATConv semantics) on 8 Trainium2 NeuronCores.

Strategy (graph-parallel, per sharding hint):
  - Host load-balances dst nodes into 8*49 windows of 128 dsts (LPT packing)
    so every core/window has near-equal edge counts; node order is permuted
    accordingly and the output inverse-permuted at the end.
  - Per layer: each core projects its own nodes ([feat|el|er] in one matmul,
    since el = (h@W)@al_diag = h@(W@al_diag)), packs [feat_bf16|el_f32] into a
    768B G-row, AllGathers G across cores; er goes to a local 256B-padded
    per-node table.
  - Edge phase per 128-dst window: dma_gather the edges' source G-rows
    (edges pre-sorted by dst into windows; int16 gather indices handled by a
    lo/hi source-offset split), dma_gather er per edge from the padded er
    table (indices = dst rows), compute ex=exp(LeakyReLU(el+er)) batched,
    build the edge->dst one-hot with one batched iota-compare, weight gathered
    feats by ex in one batched multiply, then one PE matmul per 128-edge tile
    accumulates the unnormalized aggregation and one more the softmax
    denominator into PSUM (normalization divides at the end; segment-max is
    unnecessary because logits are small and exp cannot overflow fp32).
"""

from contextlib import ExitStack

import numpy as np

import concourse.bass as bass  # noqa: F401
import concourse.bacc as bacc
import concourse.mybir as mybir
import concourse.tile as tile
from concourse import bass_utils

F32 = mybir.dt.float32
BF16 = mybir.dt.bfloat16
I16 = mybir.dt.int16

GROW = 384  # bf16 elements per G row: [feat 256 | el-as-f32 8 | pad] = 768B
ERW = 64    # f32 elements per padded er row (256B)
GMAX = 1024  # max indices per dma_gather call with single_packet=True


class Cfg:
    def __init__(self, n, e, fin, h, dh, ncores, wpc, lo_rows=32768):
        self.N, self.E, self.FIN, self.H, self.DH = n, e, fin, h, dh
        self.HID = h * dh
        self.NCORES, self.WPC = ncores, wpc
        self.NPC = wpc * 128            # padded nodes per core
        self.NPAD = ncores * self.NPC   # padded global nodes
        assert self.NPAD >= n
        self.LO_ROWS = min(lo_rows, self.NPAD)
        self.HI_OFF = max(self.NPAD - self.LO_ROWS, 0)
        self.KT = self.HID // 128       # K tiles for layers 1,2
        self.KT0 = fin // 128           # K tiles for layer 0


def _lpt_windows(deg, cfg):
    """Assign nodes to ncores*wpc windows of exactly 128 slots, balancing
    per-window edge counts (LPT greedy). Returns perm_pos[node] -> slot."""
    import heapq

    nw = cfg.NCORES * cfg.WPC
    order = np.argsort(-deg, kind="stable")
    heap = [(0, w) for w in range(nw)]
    heapq.heapify(heap)
    counts = np.zeros(nw, np.int64)
    wsum = np.zeros(nw, np.int64)
    assign = np.empty(cfg.N, np.int64)
    for n in order:
        while True:
            s, w = heapq.heappop(heap)
            if counts[w] < 128:
                break
        assign[n] = w
        counts[w] += 1
        wsum[w] += deg[n]
        if counts[w] < 128:
            heapq.heappush(heap, (wsum[w], w))
    perm_pos = np.empty(cfg.N, np.int64)
    next_row = np.zeros(nw, np.int64)
    for n in range(cfg.N):
        w = assign[n]
        perm_pos[n] = w * 128 + next_row[w]
        next_row[w] += 1
    return perm_pos


def _wrap16(vals, cap):
    """Wrap a list of idx values into the Q7 [16, cap//16] layout."""
    out = np.zeros((16, cap // 16), np.int16)
    j = np.arange(len(vals))
    out[j % 16, j // 16] = vals.astype(np.int16)
    return out


def preprocess(x, edge_index, cfg):
    src = np.asarray(edge_index[0], dtype=np.int64)
    dst = np.asarray(edge_index[1], dtype=np.int64)
    deg = np.bincount(dst, minlength=cfg.N)
    perm_pos = _lpt_windows(deg, cfg)

    psrc = perm_pos[src]
    pdst = perm_pos[dst]
    gw = pdst // 128
    row = pdst % 128

    ncores, wpc = cfg.NCORES, cfg.WPC
    core = gw // wpc
    wi = gw % wpc
    is_lo = psrc < cfg.LO_ROWS

    ek = (core * wpc + wi) * 2 + (~is_lo).astype(np.int64)
    sort_idx = np.lexsort((row, ek))
    s_psrc = psrc[sort_idx]
    s_row = row[sort_idx]
    s_ek = ek[sort_idx]
    grp_start = np.searchsorted(s_ek, np.arange(ncores * wpc * 2 + 1))

    lo_t = np.zeros(wpc, np.int64)
    hi_t = np.zeros(wpc, np.int64)
    for w in range(wpc):
        for c in range(ncores):
            g = (c * wpc + w) * 2
            nlo = grp_start[g + 1] - grp_start[g]
            nhi = grp_start[g + 2] - grp_start[g + 1]
            lo_t[w] = max(lo_t[w], (nlo + 127) // 128)
            hi_t[w] = max(hi_t[w], (nhi + 127) // 128)
    T = lo_t + hi_t
    TT = int(T.sum())
    toff = np.concatenate([[0], np.cumsum(T)]).astype(np.int64)
    nidx = TT * 8

    idx16 = np.zeros((ncores, 16, nidx), np.int16)
    dstrow = np.full((ncores, 128, TT), -1, np.float32)
    sten = np.zeros((ncores, 128, TT, 2), np.float32)

    for c in range(ncores):
        for w in range(wpc):
            base_t = toff[w]
            for reg in (0, 1):
                g = (c * wpc + w) * 2 + reg
                lo, hi_ = grp_start[g], grp_start[g + 1]
                e_ps = s_psrc[lo:hi_]
                e_r = s_row[lo:hi_]
                nt = lo_t[w] if reg == 0 else hi_t[w]
                if nt == 0:
                    assert len(e_ps) == 0
                    continue
                cap = nt * 128
                n_e = len(e_ps)
                vals = np.zeros(cap, np.int64)
                vals[:n_e] = e_ps if reg == 0 else e_ps - cfg.HI_OFF
                assert (vals >= 0).all() and (vals < cfg.LO_ROWS).all()
                rt0 = base_t + (0 if reg == 0 else lo_t[w])
                idx16[c, :, rt0 * 8:(rt0 + nt) * 8] = _wrap16(vals, cap)
                rows = np.full(cap, -1, np.int64)
                rows[:n_e] = e_r
                dstrow[c, :, rt0:rt0 + nt] = rows.reshape(nt, 128).T
                seg = np.searchsorted(e_r, np.arange(129))
                for tl in range(nt):
                    b = tl * 128
                    sten[c, :, rt0 + tl, 0] = np.clip(seg[:-1] - b, 0, 128)
                    sten[c, :, rt0 + tl, 1] = np.clip(seg[1:] - b, 0, 128)

    xT = np.zeros((ncores, cfg.FIN, cfg.NPC), np.float32)
    inv_rows = np.full(cfg.NPAD, -1, np.int64)
    inv_rows[perm_pos] = np.arange(cfg.N)
    xf = np.asarray(x, np.float32)
    for c in range(ncores):
        sl = inv_rows[c * cfg.NPC:(c + 1) * cfg.NPC]
        valid = sl >= 0
        xc = np.zeros((cfg.NPC, cfg.FIN), np.float32)
        xc[valid] = xf[sl[valid]]
        xT[c] = xc.T

    # HW Q7 cores each read their own 16-partition group: replicate.
    idx16 = np.tile(idx16, (1, 8, 1))

    return dict(perm_pos=perm_pos, lo_t=lo_t, hi_t=hi_t, T=T, TT=TT,
                toff=toff, nidx=nidx, idx16=idx16,
                dstrow=dstrow, sten=sten, xT=xT)


def pack_weights(cfg, Ws, als, ars, bs, resW0):
    kts = []
    for l, W in enumerate(Ws):
        ALf = np.zeros((cfg.HID, cfg.H), np.float32)
        ARf = np.zeros((cfg.HID, cfg.H), np.float32)
        for h in range(cfg.H):
            ALf[h * cfg.DH:(h + 1) * cfg.DH, h] = als[l][h]
            ARf[h * cfg.DH:(h + 1) * cfg.DH, h] = ars[l][h]
        Wc = np.concatenate([W, W @ ALf, W @ ARf], axis=1)  # [fin, 264]
        fin = W.shape[0]
        for k in range(fin // 128):
            kts.append(Wc[k * 128:(k + 1) * 128])
    w_all = np.transpose(np.stack(kts), (1, 0, 2)).copy()  # [128, nk, 264]
    b_rep = np.stack([np.tile(b[None, :], (128, 1)) for b in bs], axis=1)
    return w_all.astype(np.float32), b_rep.astype(np.float32), \
        resW0.astype(np.float32)


def _gather(nc, out_ap, in_ap, idx_sb, col0, n, elem, q=0):
    """One dma_gather call for n indices (single_packet only up to the
    64-desc/engine packet ceiling). q selects the SWDGE queue: queue q's
    descriptors are emitted by Q7 core pair (2q, 2q+1), so spreading
    windows across queues parallelizes descriptor generation 4x."""
    nc.gpsimd.dma_gather(
        out_ap=out_ap, in_ap=in_ap,
        idxs_ap=idx_sb[:, col0: col0 + n // 16],
        num_idxs=n, num_idxs_reg=n, elem_size=elem,
        single_packet=(n <= GMAX), queue_num=q)


def build_program(cfg, meta, num_cores):
    nc = bacc.Bacc("TRN2", target_bir_lowering=False, debug=False,
                   num_devices=num_cores, num_swdge_queues=4)
    NPC, HID = cfg.NPC, cfg.HID
    WPC = cfg.WPC
    NK = cfg.KT0 + 2 * cfg.KT
    lo_t, hi_t, T, toff, TT = meta["lo_t"], meta["hi_t"], meta["T"], \
        meta["toff"], meta["TT"]

    d_xT = nc.dram_tensor("xT", [cfg.FIN, NPC], F32, kind="ExternalInput")
    d_w = nc.dram_tensor("w_all", [128, NK, 264], F32, kind="ExternalInput")
    d_rw = nc.dram_tensor("resW0", [128, 256], F32, kind="ExternalInput")
    d_b = nc.dram_tensor("b_rep", [128, 3, 256], F32, kind="ExternalInput")
    d_id = nc.dram_tensor("ident", [128, 128], F32, kind="ExternalInput")
    d_idx = nc.dram_tensor("idx16", [128, meta["nidx"]], I16,
                           kind="ExternalInput")
    d_dr = nc.dram_tensor("dstrow", [128, TT], F32, kind="ExternalInput")
    d_st = nc.dram_tensor("sten", [128, TT, 2], F32, kind="ExternalInput")
    d_out = nc.dram_tensor("out", [NPC, cfg.DH], F32, kind="ExternalOutput")

    maxT = int(T.max())

    with ExitStack() as ctx:
        tc = ctx.enter_context(tile.TileContext(nc))
        cpool = ctx.enter_context(tc.tile_pool(name="const", bufs=1))
        dram = ctx.enter_context(tc.tile_pool(name="dram", bufs=1,
                                              space="DRAM"))
        fgpool = ctx.enter_context(tc.tile_pool(name="fg", bufs=4))
        ohpool = ctx.enter_context(tc.tile_pool(name="oh", bufs=2))
        epool = ctx.enter_context(tc.tile_pool(name="e", bufs=3))
        wpool = ctx.enter_context(tc.tile_pool(name="wt", bufs=2))
        hpool = ctx.enter_context(tc.tile_pool(name="h", bufs=4))
        gpool = ctx.enter_context(tc.tile_pool(name="g", bufs=3))
        ps_m = ctx.enter_context(tc.tile_pool(name="psm", bufs=2,
                                              space="PSUM"))
        ps_p = ctx.enter_context(tc.tile_pool(name="psp", bufs=3,
                                              space="PSUM"))
        ps_e = ctx.enter_context(tc.tile_pool(name="pse", bufs=2,
                                              space="PSUM"))

        g_loc = dram.tile([NPC, GROW], BF16)
        g_fulls = [
            dram.tile([cfg.NPAD, GROW], BF16, name=f"g_full{i}",
                      addr_space="Shared" if num_cores > 4 else "Local")
            for i in range(3)]
        hbuf = [dram.tile([NPC, HID], F32, name="hbuf0"),
                dram.tile([NPC, HID], F32, name="hbuf1")]
        res0 = dram.tile([NPC, HID], F32)

        w_sb = cpool.tile([128, NK, 264], F32)
        nc.sync.dma_start(w_sb[:], d_w[:])
        rw_sb = cpool.tile([128, 256], F32)
        nc.sync.dma_start(rw_sb[:], d_rw[:])
        b_sb = cpool.tile([128, 3, 256], F32)
        nc.sync.dma_start(b_sb[:], d_b[:])
        id_sb = cpool.tile([128, 128], F32)
        nc.sync.dma_start(id_sb[:], d_id[:])
        idx_sb = cpool.tile([128, meta["nidx"]], I16)
        nc.sync.dma_start(idx_sb[:], d_idx[:])
        dr_sb = cpool.tile([128, TT], F32)
        nc.sync.dma_start(dr_sb[:], d_dr[:])
        st_sb = cpool.tile([128, TT, 2], F32)
        nc.sync.dma_start(st_sb[:], d_st[:])
        er_res = cpool.tile([128, WPC, 4], BF16)
        iota_sb = cpool.tile([128, 128], F32)
        nc.gpsimd.iota(iota_sb[:], pattern=[[1, 128]], base=0,
                       channel_multiplier=0,
                       allow_small_or_imprecise_dtypes=True)

        kt_of_layer = [list(range(cfg.KT0)),
                       list(range(cfg.KT0, cfg.KT0 + cfg.KT)),
                       list(range(cfg.KT0 + cfg.KT, NK))]

        for l in range(3):
            # ---------------- projection phase ----------------
            for nt in range(WPC):
                kts = kt_of_layer[l]
                lhsTs = []
                if l == 0:
                    xt = hpool.tile([128, 128], F32, tag="lhsT")
                    nc.sync.dma_start(xt[:], d_xT[:, nt * 128:(nt + 1) * 128])
                    lhsTs.append(xt)
                else:
                    h_in = hpool.tile([128, HID], F32, tag="hin")
                    nc.sync.dma_start(
                        h_in[:], hbuf[(l + 1) % 2][nt * 128:(nt + 1) * 128, :])
                    for ft in range(cfg.KT):
                        pst = ps_p.tile([128, 128], F32, tag="pp")
                        nc.tensor.transpose(
                            pst[:], h_in[:, ft * 128:(ft + 1) * 128], id_sb[:])
                        hT = hpool.tile([128, 128], F32, tag="lhsT")
                        nc.scalar.copy(hT[:], pst[:])
                        lhsTs.append(hT)
                pp = ps_p.tile([128, 264], F32, tag="pp")
                for k, (kt, lt) in enumerate(zip(kts, lhsTs)):
                    nc.tensor.matmul(pp[:], lt[:], w_sb[:, kt, :],
                                     start=(k == 0), stop=(k == len(kts) - 1))
                g_sb = gpool.tile([128, 264], BF16)
                nc.scalar.copy(g_sb[:, 0:256], pp[:, 0:256])
                nc.scalar.copy(g_sb[:, 256:264].bitcast(F32), pp[:, 256:260])
                nc.scalar.copy(er_res[:, nt, :], pp[:, 260:264])
                nc.sync.dma_start(g_loc[nt * 128:(nt + 1) * 128, 0:264],
                                  g_sb[:, 0:264])
                if l == 0:
                    pr = ps_p.tile([128, 256], F32, tag="pp")
                    nc.tensor.matmul(pr[:], lhsTs[0][:], rw_sb[:],
                                     start=True, stop=True)
                    r_sb = gpool.tile([128, 256], F32, tag="res")
                    nc.scalar.copy(r_sb[:], pr[:])
                    nc.sync.dma_start(res0[nt * 128:(nt + 1) * 128, :],
                                      r_sb[:])

            g_full = g_fulls[l]
            nc.gpsimd.collective_compute(
                "AllGather", mybir.AluOpType.bypass,
                replica_groups=[list(range(num_cores))],
                ins=[g_loc.opt()], outs=[g_full.opt()])

            # ---------------- aggregation phase ----------------
            for w in range(WPC):
                Tw = int(T[w])
                lt_, ht_ = int(lo_t[w]), int(hi_t[w])
                t0 = int(toff[w])
                fg = fgpool.tile([128, maxT, GROW], BF16)
                if lt_ > 0:
                    _gather(nc, fg[:, 0:lt_, :], g_full[0:cfg.LO_ROWS, :],
                            idx_sb, t0 * 8, lt_ * 128, GROW, q=w % 4)
                if ht_ > 0:
                    _gather(nc, fg[:, lt_:Tw, :],
                            g_full[cfg.HI_OFF:cfg.NPAD, :], idx_sb,
                            (t0 + lt_) * 8, ht_ * 128, GROW, q=w % 4)

                # staircase one-hot (dsts-on-partitions), batched:
                # mst[d, t, e] = (st[d,t] <= e < en[d,t])
                mst = ohpool.tile([128, maxT, 128], BF16, tag="mst")
                iota3 = iota_sb[:].unsqueeze(1).broadcast_to([128, Tw, 128])
                nc.vector.tensor_tensor(
                    mst[:, 0:Tw, :], iota3,
                    st_sb[:, t0:t0 + Tw, 0:1].broadcast_to([128, Tw, 128]),
                    mybir.AluOpType.is_ge)
                mlt = ohpool.tile([128, maxT, 128], BF16, tag="mlt")
                nc.vector.tensor_tensor(
                    mlt[:, 0:Tw, :], iota3,
                    st_sb[:, t0:t0 + Tw, 1:2].broadcast_to([128, Tw, 128]),
                    mybir.AluOpType.is_lt)
                nc.vector.tensor_tensor(
                    mst[:, 0:Tw, :], mst[:, 0:Tw, :], mlt[:, 0:Tw, :],
                    mybir.AluOpType.mult)
                pe_er = ps_e.tile([128, maxT, 4], F32)
                for t in range(Tw):
                    nc.tensor.matmul(pe_er[:, t, :], mst[:, t, :],
                                     er_res[:, w, :], start=True, stop=True)

                # one-hot (edges-on-partitions): mt[e, t, d] = (row_e,t == d)
                mt = ohpool.tile([128, maxT, 128], BF16)
                nc.vector.tensor_tensor(
                    mt[:, 0:Tw, :],
                    dr_sb[:, t0:t0 + Tw].unsqueeze(2)
                         .broadcast_to([128, Tw, 128]),
                    iota_sb[:].unsqueeze(1).broadcast_to([128, Tw, 128]),
                    mybir.AluOpType.is_equal)

                # logits, batched over the window
                el_v = fg[:, 0:Tw, 256:264].bitcast(F32)    # [128, Tw, 4]
                e_sb = epool.tile([128, maxT, 4], F32, tag="e")
                nc.vector.tensor_tensor(e_sb[:, 0:Tw, :], el_v,
                                        pe_er[:, 0:Tw, :],
                                        mybir.AluOpType.add)
                nc.vector.scalar_tensor_tensor(
                    e_sb[:, 0:Tw, :], e_sb[:, 0:Tw, :], 0.2, e_sb[:, 0:Tw, :],
                    mybir.AluOpType.mult, mybir.AluOpType.max)
                ex_sb = epool.tile([128, maxT, 4], F32, tag="ex")
                nc.scalar.activation(ex_sb[:, 0:Tw, :], e_sb[:, 0:Tw, :],
                                     mybir.ActivationFunctionType.Exp)
                exb = epool.tile([128, maxT, 4], BF16, tag="exb")
                nc.vector.tensor_copy(exb[:, 0:Tw, :], ex_sb[:, 0:Tw, :])

                # weighted feats, batched: wsb[e,t,h,d] = fg[e,t,h,d]*ex[e,t,h]
                wsb = wpool.tile([128, maxT, 256], BF16)
                nc.vector.tensor_tensor(
                    wsb[:, 0:Tw, :].rearrange("p t (h d) -> p t h d", h=4),
                    fg[:, 0:Tw, 0:256].rearrange("p t (h d) -> p t h d", h=4),
                    exb[:, 0:Tw, :].unsqueeze(3).broadcast_to(
                        [128, Tw, 4, 64]),
                    mybir.AluOpType.mult)

                pm = ps_m.tile([128, 260], F32)
                for t in range(Tw):
                    nc.tensor.matmul(pm[:, 0:256], mt[:, t, :], wsb[:, t, :],
                                     start=(t == 0), stop=(t == Tw - 1),
                                     skip_group_check=True)
                for t in range(Tw):
                    nc.tensor.matmul(pm[:, 256:260], mt[:, t, :],
                                     exb[:, t, :],
                                     start=(t == 0), stop=(t == Tw - 1),
                                     skip_group_check=True)

                den = epool.tile([128, 4], F32, tag="den")
                nc.vector.tensor_scalar(den[:], pm[:, 256:260], 1e-16, None,
                                        mybir.AluOpType.max)
                rden = epool.tile([128, 4], F32, tag="rden")
                nc.vector.reciprocal(rden[:], den[:])
                hn = hpool.tile([128, HID], F32, tag="hn")
                nc.vector.tensor_tensor(
                    hn[:].rearrange("p (h d) -> p h d", h=4),
                    pm[:, 0:256].rearrange("p (h d) -> p h d", h=4),
                    rden[:].unsqueeze(2).broadcast_to([128, 4, 64]),
                    mybir.AluOpType.mult)
                rsb = hpool.tile([128, HID], F32, tag="res_in")
                rsrc = res0 if l == 0 else hbuf[(l + 1) % 2]
                nc.sync.dma_start(rsb[:], rsrc[w * 128:(w + 1) * 128, :])
                nc.vector.tensor_tensor(hn[:], hn[:], rsb[:],
                                        mybir.AluOpType.add)
                nc.vector.tensor_tensor(hn[:], hn[:], b_sb[:, l, :],
                                        mybir.AluOpType.add)
                if l < 2:
                    nc.scalar.activation(hn[:], hn[:],
                                         mybir.ActivationFunctionType.Relu)
                    nc.sync.dma_start(hbuf[l % 2][w * 128:(w + 1) * 128, :],
                                      hn[:])
                else:
                    osb = hpool.tile([128, cfg.DH], F32, tag="osb")
                    nc.vector.tensor_reduce(
                        osb[:],
                        hn[:].rearrange("p (h d) -> p d h", h=4),
                        mybir.AxisListType.X, mybir.AluOpType.add)
                    nc.vector.tensor_scalar(osb[:], osb[:], 1.0 / cfg.H, None,
                                            mybir.AluOpType.mult)
                    nc.sync.dma_start(d_out[w * 128:(w + 1) * 128, :], osb[:])

    nc.compile()
    return nc


def make_in_maps(cfg, meta, wnp, num_cores):
    w_all, b_rep, rw = wnp
    ident = np.eye(128, dtype=np.float32)
    maps = []
    for c in range(num_cores):
        maps.append({
            "xT": meta["xT"][c],
            "w_all": w_all, "resW0": rw, "b_rep": b_rep, "ident": ident,
            "idx16": meta["idx16"][c],
            "dstrow": meta["dstrow"][c],
            "sten": meta["sten"][c],
        })
    return maps


def assemble_output(cfg, meta, results):
    out = np.empty((cfg.N, cfg.DH), np.float32)
    full = np.concatenate([r["out"] for r in results], axis=0)
    out[:] = full[meta["perm_pos"]]
    return out


def solve(x, edge_index, Ws, als, ars, bs, resW0, cfg, trace=False):
    meta = preprocess(x, edge_index, cfg)
    wnp = pack_weights(cfg, Ws, als, ars, bs, resW0)
    nc = build_program(cfg, meta, cfg.NCORES)
    in_maps = make_in_maps(cfg, meta, wnp, cfg.NCORES)
    res = bass_utils.run_bass_kernel_spmd(
        nc, in_maps, core_ids=list(range(cfg.NCORES)), trace=trace)
    out = assemble_output(cfg, meta, res.results)
    return out, res


def kernel(x, edge_index, W0, W1, W2, al0, al1, al2, ar0, ar1, ar2,
           b0, b1, b2, resW0):
    cfg = Cfg(n=50000, e=800000, fin=128, h=4, dh=64, ncores=8, wpc=49)
    out, _ = solve(np.asarray(x, np.float32), np.asarray(edge_index),
                   [np.asarray(W0, np.float32), np.asarray(W1, np.float32),
                    np.asarray(W2, np.float32)],
                   [np.asarray(al0, np.float32), np.asarray(al1, np.float32),
                    np.asarray(al2, np.float32)],
                   [np.asarray(ar0, np.float32), np.asarray(ar1, np.float32),
                    np.asarray(ar2, np.float32)],
                   [np.asarray(b0, np.float32), np.asarray(b1, np.float32),
                    np.asarray(b2, np.float32)],
                   np.asarray(resW0, np.float32), cfg)
    return out



# revision 29
# speedup vs baseline: 1.3390x; 1.1108x over previous
"""3-layer GAT (DGL GATConv semantics) on 8 Trainium2 NeuronCores.

v2 strategy (graph-parallel; see sim_v2.py for the numpy golden model):
  - Host load-balances dst nodes into 8*49 windows of 128 dsts (LPT packing);
    node order is permuted accordingly, output inverse-permuted at the end.
  - Per layer: each core projects its local nodes ([feat|el|er] in one bf16
    matmul), packs [feat bf16 | el f32] into a 768B G-row plus one dedicated
    pad row per core (feat=0, el=-60 so padded edges vanish in exp), and
    AllGathers G.
  - Edge phase per 128-dst window: dma_gather source G-rows (edges presorted
    by dst; 4 SWDGE queues round-robin so descriptor emission runs on all
    four Q7 core pairs), build ONE is_ge staircase mask B per tile, and use
    it twice with suffix-sum telescoping:
      * er per edge = B.T @ diff(er) (first-difference along dst rows,
        bf16 hi/lo split for accuracy),
      * per-dst segments = first-difference (bidiagonal matmul) of the
        suffix sums B.T-accumulated over tiles, with the softmax denominator
        fused as 4 extra matmul columns and the 1e-16 clamp as a rank-1
        epsilon matmul.
    LeakyReLU/exp/copies run on the Scalar engine; the Vector engine only
    does the mask build, logit add, weighted-feature multiply, and the
    normalization arithmetic. h stays resident in SBUF (bf16) across layers
    (it is both the next layer's input and its residual).
"""

from contextlib import ExitStack

import numpy as np

import concourse.bass as bass  # noqa: F401
import concourse.bacc as bacc
import concourse.mybir as mybir
import concourse.tile as tile
from concourse import bass_utils

F32 = mybir.dt.float32
BF16 = mybir.dt.bfloat16
I16 = mybir.dt.int16

GROW = 384  # bf16 elements per G row: [feat 256 | el-as-f32 8 | pad] = 768B
GMAX = 1024


def _bf16(a):
    import ml_dtypes
    return np.asarray(a, np.float32).astype(ml_dtypes.bfloat16)


class Cfg:
    def __init__(self, n, e, fin, h, dh, ncores, wpc, lo_rows=32768):
        self.N, self.E, self.FIN, self.H, self.DH = n, e, fin, h, dh
        self.HID = h * dh
        self.NCORES, self.WPC = ncores, wpc
        self.NPC = wpc * 128            # nodes per core (excl. pad row)
        self.NPC1 = self.NPC + 1        # + dedicated pad G-row
        self.NPAD = ncores * self.NPC   # padded global nodes
        self.NPAD8 = ncores * self.NPC1  # G-table rows incl. pad rows
        assert self.NPAD >= n
        self.LO_ROWS = min(lo_rows, self.NPAD8)
        self.HI_OFF = max(self.NPAD8 - self.LO_ROWS, 0)
        self.KT = self.HID // 128       # K tiles for layers 1,2
        self.KT0 = fin // 128           # K tiles for layer 0
        self.PAD_LO = self.NPC          # core 0's pad row (< LO_ROWS)
        self.PAD_HI = self.NPAD8 - 1 - self.HI_OFF  # core 7's pad row - HI_OFF


def _lpt_windows(deg, cfg):
    """Assign nodes to ncores*wpc windows of exactly 128 slots, balancing
    per-window edge counts (LPT greedy). Returns perm_pos[node] -> slot."""
    import heapq

    nw = cfg.NCORES * cfg.WPC
    order = np.argsort(-deg, kind="stable")
    heap = [(0, w) for w in range(nw)]
    heapq.heapify(heap)
    counts = np.zeros(nw, np.int64)
    wsum = np.zeros(nw, np.int64)
    assign = np.empty(cfg.N, np.int64)
    for n in order:
        while True:
            s, w = heapq.heappop(heap)
            if counts[w] < 128:
                break
        assign[n] = w
        counts[w] += 1
        wsum[w] += deg[n]
        if counts[w] < 128:
            heapq.heappush(heap, (wsum[w], w))
    perm_pos = np.empty(cfg.N, np.int64)
    next_row = np.zeros(nw, np.int64)
    for n in range(cfg.N):
        w = assign[n]
        perm_pos[n] = w * 128 + next_row[w]
        next_row[w] += 1
    return perm_pos


def _wrap16(vals, cap):
    """Wrap a list of idx values into the Q7 [16, cap//16] layout."""
    out = np.zeros((16, cap // 16), np.int16)
    j = np.arange(len(vals))
    out[j % 16, j // 16] = vals.astype(np.int16)
    return out


def preprocess(x, edge_index, cfg):
    src = np.asarray(edge_index[0], dtype=np.int64)
    dst = np.asarray(edge_index[1], dtype=np.int64)
    deg = np.bincount(dst, minlength=cfg.N)
    perm_pos = _lpt_windows(deg, cfg)

    psrc = perm_pos[src]
    pdst = perm_pos[dst]
    g_of = (psrc // cfg.NPC) * cfg.NPC1 + (psrc % cfg.NPC)  # G-table rows
    gw = pdst // 128
    row = pdst % 128

    ncores, wpc = cfg.NCORES, cfg.WPC
    core = gw // wpc
    wi = gw % wpc
    is_lo = g_of < cfg.LO_ROWS

    ek = (core * wpc + wi) * 2 + (~is_lo).astype(np.int64)
    sort_idx = np.lexsort((row, ek))
    s_g = g_of[sort_idx]
    s_row = row[sort_idx]
    s_ek = ek[sort_idx]
    grp_start = np.searchsorted(s_ek, np.arange(ncores * wpc * 2 + 1))

    lo_t = np.zeros(wpc, np.int64)
    hi_t = np.zeros(wpc, np.int64)
    for w in range(wpc):
        for c in range(ncores):
            g = (c * wpc + w) * 2
            nlo = grp_start[g + 1] - grp_start[g]
            nhi = grp_start[g + 2] - grp_start[g + 1]
            lo_t[w] = max(lo_t[w], (nlo + 127) // 128)
            hi_t[w] = max(hi_t[w], (nhi + 127) // 128)
    T = lo_t + hi_t
    TT = int(T.sum())
    toff = np.concatenate([[0], np.cumsum(T)]).astype(np.int64)
    nidx = TT * 8

    idx16 = np.zeros((ncores, 16, nidx), np.int16)
    # st defaults to 128 so tiles a window reserves beyond this core's own
    # group tile count get an all-zero mask (B = e >= 128 never true).
    st = np.full((ncores, 128, TT), 128.0, np.float32)
    # dstrow defaults to -1 so unwritten tiles get an all-zero edge mask
    # (d <= dstrow never true); pad edge slots also use -1.
    dstrow = np.full((ncores, 128, TT), -1.0, np.float32)

    for c in range(ncores):
        for w in range(wpc):
            base_t = toff[w]
            for reg in (0, 1):
                g = (c * wpc + w) * 2 + reg
                lo, hi_ = grp_start[g], grp_start[g + 1]
                e_g = s_g[lo:hi_]
                e_r = s_row[lo:hi_]
                nt = lo_t[w] if reg == 0 else hi_t[w]
                if nt == 0:
                    assert len(e_g) == 0
                    continue
                cap = nt * 128
                n_e = len(e_g)
                pad = cfg.PAD_LO if reg == 0 else cfg.PAD_HI
                vals = np.full(cap, pad, np.int64)
                vals[:n_e] = e_g if reg == 0 else e_g - cfg.HI_OFF
                assert (vals >= 0).all() and (vals < cfg.LO_ROWS).all()
                rt0 = base_t + (0 if reg == 0 else lo_t[w])
                idx16[c, :, rt0 * 8:(rt0 + nt) * 8] = _wrap16(vals, cap)
                rows = np.full(cap, -1, np.int64)
                rows[:n_e] = e_r
                dstrow[c, :, rt0:rt0 + nt] = rows.reshape(nt, 128).T
                seg = np.searchsorted(e_r, np.arange(129))
                for tl in range(nt):
                    b = tl * 128
                    st[c, :, rt0 + tl] = np.clip(seg[:-1] - b, 0, 128)

    xT = np.zeros((ncores, cfg.FIN, cfg.NPC), np.float32)
    inv_rows = np.full(cfg.NPAD, -1, np.int64)
    inv_rows[perm_pos] = np.arange(cfg.N)
    xf = np.asarray(x, np.float32)
    for c in range(ncores):
        sl = inv_rows[c * cfg.NPC:(c + 1) * cfg.NPC]
        valid = sl >= 0
        xc = np.zeros((cfg.NPC, cfg.FIN), np.float32)
        xc[valid] = xf[sl[valid]]
        xT[c] = xc.T

    # HW Q7 cores each read their own 16-partition group: replicate.
    idx16 = np.tile(idx16, (1, 8, 1))

    return dict(perm_pos=perm_pos, lo_t=lo_t, hi_t=hi_t, T=T, TT=TT,
                toff=toff, nidx=nidx, idx16=idx16, st=st, dstrow=dstrow,
                xT=xT)


def pack_weights(cfg, Ws, als, ars, bs, resW0):
    kts = []
    for l, W in enumerate(Ws):
        ALf = np.zeros((cfg.HID, cfg.H), np.float32)
        ARf = np.zeros((cfg.HID, cfg.H), np.float32)
        for h in range(cfg.H):
            ALf[h * cfg.DH:(h + 1) * cfg.DH, h] = als[l][h]
            ARf[h * cfg.DH:(h + 1) * cfg.DH, h] = ars[l][h]
        Wc = np.concatenate([W, W @ ALf, W @ ARf], axis=1)  # [fin, 264]
        fin = W.shape[0]
        for k in range(fin // 128):
            kts.append(Wc[k * 128:(k + 1) * 128])
    w_all = np.transpose(np.stack(kts), (1, 0, 2)).copy()  # [128, nk, 264]
    b_rep = np.stack([np.tile(b[None, :], (128, 1)) for b in bs], axis=1)
    has_b = any(np.abs(b).max() > 0 for b in bs)
    return w_all, b_rep, np.asarray(resW0, np.float32), has_b


def _consts(cfg):
    # D: pm2[d] = pm[d] - pm[d+1]  (lhsT[k, d]: +1 at k=d, -1 at k=d+1)
    D = np.zeros((128, 128), np.float32)
    Dr = np.zeros((128, 128), np.float32)
    for d in range(128):
        D[d, d] = 1.0
        if d + 1 < 128:
            D[d + 1, d] = -1.0
        # Dr: er_diff[d] = er[d] - er[d-1]
        Dr[d, d] = 1.0
        if d - 1 >= 0:
            Dr[d - 1, d] = -1.0
    ones = np.ones((1, 128), np.float32)
    eps = np.zeros((1, 264), np.float32)
    eps[0, 256:260] = 1e-16
    # pad G-row: feat 0, el = -60.0 (4x f32) at bf16 cols 256:264
    pad = np.zeros(384, np.uint16)
    el = np.full(4, -60.0, np.float32)
    pad[256:264] = el.view(np.uint16)
    import ml_dtypes
    padrow = pad.view(ml_dtypes.bfloat16).reshape(1, 384)
    return D, Dr, ones, eps, padrow


def _gather(nc, out_ap, in_ap, idx_sb, col0, n, elem, q=0):
    """One dma_gather call for n indices. q selects the SWDGE queue: queue
    q's descriptors are emitted by Q7 core pair (2q, 2q+1), so spreading
    windows across queues parallelizes descriptor generation 4x."""
    nc.gpsimd.dma_gather(
        out_ap=out_ap, in_ap=in_ap,
        idxs_ap=idx_sb[:, col0: col0 + n // 16],
        num_idxs=n, num_idxs_reg=n, elem_size=elem,
        single_packet=(n <= GMAX), queue_num=q)


def build_program(cfg, meta, num_cores, has_b=False, dbg_layer=None):
    nc = bacc.Bacc("TRN2", target_bir_lowering=False, debug=False,
                   num_devices=num_cores, num_swdge_queues=4)
    NPC, HID = cfg.NPC, cfg.HID
    WPC = cfg.WPC
    NK = cfg.KT0 + 2 * cfg.KT
    lo_t, hi_t, T, toff, TT = meta["lo_t"], meta["hi_t"], meta["T"], \
        meta["toff"], meta["TT"]
    maxT = int(T.max())
    AF = mybir.ActivationFunctionType

    d_xT = nc.dram_tensor("xT", [cfg.FIN, NPC], BF16, kind="ExternalInput")
    d_w = nc.dram_tensor("w_all", [128, NK, 264], BF16, kind="ExternalInput")
    d_rw = nc.dram_tensor("resW0", [128, 256], BF16, kind="ExternalInput")
    d_id = nc.dram_tensor("ident", [128, 128], BF16, kind="ExternalInput")
    d_D = nc.dram_tensor("Dmat", [128, 128], F32, kind="ExternalInput")
    d_Dr = nc.dram_tensor("Drmat", [128, 128], F32, kind="ExternalInput")
    d_ones = nc.dram_tensor("ones1", [1, 128], F32, kind="ExternalInput")
    d_eps = nc.dram_tensor("epsrow", [1, 264], F32, kind="ExternalInput")
    d_pad = nc.dram_tensor("padrow", [1, GROW], BF16, kind="ExternalInput")
    d_idx = nc.dram_tensor("idx16", [128, meta["nidx"]], I16,
                           kind="ExternalInput")
    d_st = nc.dram_tensor("st", [128, TT], BF16, kind="ExternalInput")
    d_dr = nc.dram_tensor("dstrow", [128, TT], BF16, kind="ExternalInput")
    d_out = nc.dram_tensor("out", [NPC, cfg.DH], F32, kind="ExternalOutput")
    if has_b:
        d_b = nc.dram_tensor("b_rep", [128, 3, 256], F32,
                             kind="ExternalInput")
    if dbg_layer is not None:
        d_dbg = nc.dram_tensor("dbg", [NPC, HID], F32, kind="ExternalOutput")
        d_dbg2 = nc.dram_tensor("dbg2", [NPC, 524], F32,
                                kind="ExternalOutput")
        d_dbg3 = nc.dram_tensor("dbg3", [128, maxT * 128], F32,
                                kind="ExternalOutput")
        d_dbg4 = nc.dram_tensor("dbg4", [128, maxT * 8], F32,
                                kind="ExternalOutput")

    with ExitStack() as ctx:
        tc = ctx.enter_context(tile.TileContext(nc))
        cpool = ctx.enter_context(tc.tile_pool(name="const", bufs=1))
        dram = ctx.enter_context(tc.tile_pool(name="dram", bufs=1,
                                              space="DRAM"))
        fgpool = ctx.enter_context(tc.tile_pool(name="fg", bufs=4))
        bpool = ctx.enter_context(tc.tile_pool(name="bm", bufs=3))
        epool = ctx.enter_context(tc.tile_pool(name="e", bufs=3))
        wpool = ctx.enter_context(tc.tile_pool(name="wt", bufs=3))
        hpool = ctx.enter_context(tc.tile_pool(name="h", bufs=4))
        gpool = ctx.enter_context(tc.tile_pool(name="g", bufs=3))
        spool = ctx.enter_context(tc.tile_pool(name="s", bufs=3))
        ps_m = ctx.enter_context(tc.tile_pool(name="psm", bufs=2,
                                              space="PSUM"))
        ps_m2 = ctx.enter_context(tc.tile_pool(name="psm2", bufs=1,
                                               space="PSUM"))
        ps_p = ctx.enter_context(tc.tile_pool(name="psp", bufs=2,
                                              space="PSUM"))
        ps_t = ctx.enter_context(tc.tile_pool(name="pst", bufs=1,
                                              space="PSUM"))
        ps_e = ctx.enter_context(tc.tile_pool(name="pse", bufs=1,
                                              space="PSUM"))
        ps_q = ctx.enter_context(tc.tile_pool(name="psq", bufs=1,
                                              space="PSUM"))

        g_loc = dram.tile([cfg.NPC1, GROW], BF16)
        g_fulls = [
            dram.tile([cfg.NPAD8, GROW], BF16, name=f"g_full{i}",
                      addr_space="Shared" if num_cores > 4 else "Local")
            for i in range(3)]

        # ---- persistent SBUF state ----
        xT_sb = cpool.tile([128, NPC], BF16)
        nc.sync.dma_start(xT_sb[:], d_xT[:])
        w_sb = cpool.tile([128, NK, 264], BF16)
        nc.sync.dma_start(w_sb[:], d_w[:])
        rw_sb = cpool.tile([128, 256], BF16)
        nc.sync.dma_start(rw_sb[:], d_rw[:])
        id_sb = cpool.tile([128, 128], BF16)
        nc.sync.dma_start(id_sb[:], d_id[:])
        D_sb = cpool.tile([128, 128], F32)
        nc.sync.dma_start(D_sb[:], d_D[:])
        Dr_sb = cpool.tile([128, 128], F32)
        nc.sync.dma_start(Dr_sb[:], d_Dr[:])
        ones_sb = cpool.tile([1, 128], F32)
        nc.sync.dma_start(ones_sb[:], d_ones[:])
        eps_sb = cpool.tile([1, 264], F32)
        nc.sync.dma_start(eps_sb[:], d_eps[:])
        pad_sb = cpool.tile([1, GROW], BF16)
        nc.sync.dma_start(pad_sb[:], d_pad[:])
        idx_sb = cpool.tile([128, meta["nidx"]], I16)
        nc.sync.dma_start(idx_sb[:], d_idx[:])
        st_sb = cpool.tile([128, TT], BF16)
        nc.sync.dma_start(st_sb[:], d_st[:])
        dr_sb = cpool.tile([128, TT], BF16)
        nc.sync.dma_start(dr_sb[:], d_dr[:])
        if has_b:
            b_sb = cpool.tile([128, 3, 256], F32)
            nc.sync.dma_start(b_sb[:], d_b[:])
        er_res = cpool.tile([128, WPC, 4], F32)
        h_keep = cpool.tile([128, WPC, HID], BF16)
        iota_f = cpool.tile([128, 128], F32)
        nc.gpsimd.iota(iota_f[:], pattern=[[1, 128]], base=0,
                       channel_multiplier=0,
                       allow_small_or_imprecise_dtypes=True)
        iota_sb = cpool.tile([128, 128], BF16)
        nc.scalar.copy(iota_sb[:], iota_f[:])

        kt_of_layer = [list(range(cfg.KT0)),
                       list(range(cfg.KT0, cfg.KT0 + cfg.KT)),
                       list(range(cfg.KT0 + cfg.KT, NK))]

        for l in range(3):
            # ---------------- projection phase ----------------
            for nt in range(WPC):
                kts = kt_of_layer[l]
                lhsTs = []
                if l == 0:
                    lhsTs.append(xT_sb[:, nt * 128:(nt + 1) * 128])
                else:
                    for ft in range(cfg.KT):
                        pst = ps_t.tile([128, 128], BF16, tag="pt")
                        nc.tensor.transpose(
                            pst[:],
                            h_keep[:, nt, ft * 128:(ft + 1) * 128],
                            id_sb[:])
                        hT = hpool.tile([128, 128], BF16, tag="lhsT")
                        nc.scalar.copy(hT[:], pst[:])
                        lhsTs.append(hT[:])
                pp = ps_p.tile([128, 264], F32, tag="pp")
                for k, (kt, lt) in enumerate(zip(kts, lhsTs)):
                    nc.tensor.matmul(pp[:], lt, w_sb[:, kt, :],
                                     start=(k == 0), stop=(k == len(kts) - 1))
                g_sb = gpool.tile([128, 264], BF16)
                nc.scalar.copy(g_sb[:, 0:256], pp[:, 0:256])
                nc.scalar.copy(g_sb[:, 256:264].bitcast(F32), pp[:, 256:260])
                nc.scalar.copy(er_res[:, nt, :], pp[:, 260:264])
                nc.sync.dma_start(g_loc[nt * 128:(nt + 1) * 128, 0:264],
                                  g_sb[:, 0:264])
                if l == 0:
                    pr = ps_p.tile([128, 256], F32, tag="pp")
                    nc.tensor.matmul(pr[:], lhsTs[0], rw_sb[:],
                                     start=True, stop=True)
                    # layer-0 residual (x @ resW0) parked in h_keep
                    nc.scalar.copy(h_keep[:, nt, :], pr[:])
            nc.sync.dma_start(g_loc[NPC:NPC + 1, :], pad_sb[:])

            g_full = g_fulls[l]
            nc.gpsimd.collective_compute(
                "AllGather", mybir.AluOpType.bypass,
                replica_groups=[list(range(num_cores))],
                ins=[g_loc.opt()], outs=[g_full.opt()])

            # ---------------- aggregation phase ----------------
            for w in range(WPC):
                Tw = int(T[w])
                lt_, ht_ = int(lo_t[w]), int(hi_t[w])
                t0 = int(toff[w])

                # er first-difference with bf16 hi/lo split
                er_dq = ps_q.tile([128, 4], F32, tag="edq")
                nc.tensor.matmul(er_dq[:], Dr_sb[:], er_res[:, w, :],
                                 start=True, stop=True)
                er8 = epool.tile([128, 8], BF16, tag="er8")
                nc.scalar.copy(er8[:, 0:4], er_dq[:])
                nc.vector.tensor_tensor(er8[:, 4:8], er_dq[:], er8[:, 0:4],
                                        mybir.AluOpType.subtract)

                fg = fgpool.tile([128, maxT, GROW], BF16)
                if lt_ > 0:
                    _gather(nc, fg[:, 0:lt_, :], g_full[0:cfg.LO_ROWS, :],
                            idx_sb, t0 * 8, lt_ * 128, GROW, q=w % 4)
                if ht_ > 0:
                    _gather(nc, fg[:, lt_:Tw, :],
                            g_full[cfg.HI_OFF:cfg.NPAD8, :], idx_sb,
                            (t0 + lt_) * 8, ht_ * 128, GROW, q=w % 4)

                # dst-partitioned staircase: B[d, t, e] = (e >= st[d, t])
                # (contraction over d: the er-broadcast matmul)
                bm = bpool.tile([128, maxT, 128], BF16, tag="bmd")
                nc.vector.tensor_tensor(
                    bm[:, 0:Tw, :],
                    iota_sb[:].unsqueeze(1).broadcast_to([128, Tw, 128]),
                    st_sb[:, t0:t0 + Tw].unsqueeze(2)
                         .broadcast_to([128, Tw, 128]),
                    mybir.AluOpType.is_ge)
                # edge-partitioned dual: BT[e, t, d] = (d <= dstrow[e, t])
                # (contraction over e: the suffix-sum aggregation matmul)
                bme = bpool.tile([128, maxT, 128], BF16, tag="bme")
                nc.vector.tensor_tensor(
                    bme[:, 0:Tw, :],
                    iota_sb[:].unsqueeze(1).broadcast_to([128, Tw, 128]),
                    dr_sb[:, t0:t0 + Tw].unsqueeze(2)
                         .broadcast_to([128, Tw, 128]),
                    mybir.AluOpType.is_le)

                # er per edge via telescoping (hi+lo columns)
                pe8 = ps_e.tile([128, maxT, 8], F32, tag="pe8")
                for t in range(Tw):
                    nc.tensor.matmul(pe8[:, t, :], bm[:, t, :], er8[:],
                                     start=True, stop=True)

                # logits + LeakyReLU + exp (exp lands in wsb cols 256:260)
                el_v = fg[:, 0:Tw, 256:264].bitcast(F32)    # [128, Tw, 4]
                e_sb = epool.tile([128, maxT, 4], F32, tag="e")
                nc.vector.tensor_tensor(e_sb[:, 0:Tw, :], el_v,
                                        pe8[:, 0:Tw, 0:4],
                                        mybir.AluOpType.add)
                nc.vector.tensor_tensor(e_sb[:, 0:Tw, :], e_sb[:, 0:Tw, :],
                                        pe8[:, 0:Tw, 4:8],
                                        mybir.AluOpType.add)
                nc.vector.scalar_tensor_tensor(
                    e_sb[:, 0:Tw, :], e_sb[:, 0:Tw, :], 0.2, e_sb[:, 0:Tw, :],
                    mybir.AluOpType.mult, mybir.AluOpType.max)
                exb = epool.tile([128, maxT, 4], BF16, tag="exb")
                nc.scalar.activation(exb[:, 0:Tw, :], e_sb[:, 0:Tw, :],
                                     AF.Exp)
                wsb = wpool.tile([128, maxT, 260], BF16)
                nc.vector.tensor_copy(wsb[:, 0:Tw, 256:260], exb[:, 0:Tw, :])

                # weighted feats: wsb[e,t,h,d] = fg[e,t,h,d] * ex[e,t,h]
                nc.vector.tensor_tensor(
                    wsb[:, 0:Tw, 0:256].rearrange("p t (h d) -> p t h d",
                                                  h=4),
                    fg[:, 0:Tw, 0:256].rearrange("p t (h d) -> p t h d", h=4),
                    exb[:, 0:Tw, :].unsqueeze(3).broadcast_to(
                        [128, Tw, 4, 64]),
                    mybir.AluOpType.mult)

                # suffix sums over tiles: pm[d] = sum_t BT_t.T @ wsb_t
                pm = ps_m.tile([128, 260], F32)
                for t in range(Tw):
                    nc.tensor.matmul(pm[:], bme[:, t, :], wsb[:, t, :],
                                     start=(t == 0), stop=(t == Tw - 1),
                                     skip_group_check=True)

                # telescope to per-dst segments + epsilon clamp
                s_sb = spool.tile([128, 260], F32)
                nc.scalar.copy(s_sb[:], pm[:])
                pm2 = ps_m2.tile([128, 260], F32)
                nc.tensor.matmul(pm2[:], D_sb[:], s_sb[:],
                                 start=True, stop=False)
                nc.tensor.matmul(pm2[:], ones_sb[:], eps_sb[0:1, 0:260],
                                 start=False, stop=True)

                rden = epool.tile([128, 4], F32, tag="rden")
                nc.vector.reciprocal(rden[:], pm2[:, 256:260])
                hn = hpool.tile([128, HID], F32, tag="hn")
                nc.vector.tensor_tensor(
                    hn[:].rearrange("p (h d) -> p h d", h=4),
                    pm2[:, 0:256].rearrange("p (h d) -> p h d", h=4),
                    rden[:].unsqueeze(2).broadcast_to([128, 4, 64]),
                    mybir.AluOpType.mult)
                nc.vector.tensor_tensor(hn[:], hn[:], h_keep[:, w, :],
                                        mybir.AluOpType.add)
                if has_b:
                    nc.vector.tensor_tensor(hn[:], hn[:], b_sb[:, l, :],
                                            mybir.AluOpType.add)
                if dbg_layer is not None and l == dbg_layer and w == 0:
                    d3 = hpool.tile([128, maxT * 128], F32, tag="d3sb")
                    nc.vector.tensor_copy(
                        d3[:, 0:maxT * 4].rearrange("p (t e) -> p t e",
                                                    t=maxT),
                        fg[:, 0:maxT, 256:264].bitcast(F32))
                    nc.vector.tensor_copy(
                        d3[:, maxT * 4:maxT * 4 + maxT * 64]
                        .rearrange("p (t e) -> p t e", t=maxT),
                        fg[:, 0:maxT, 0:64])
                    nc.sync.dma_start(d_dbg3[:], d3[:])
                    d4 = hpool.tile([128, maxT * 8], F32, tag="d4sb")
                    nc.vector.tensor_copy(
                        d4[:].rearrange("p (t e) -> p t e", t=maxT),
                        pe8[:, 0:maxT, :])
                    nc.sync.dma_start(d_dbg4[:], d4[:])
                if dbg_layer is not None and l == dbg_layer:
                    dsb = hpool.tile([128, HID], F32, tag="dsb")
                    nc.vector.tensor_copy(dsb[:], hn[:])
                    nc.sync.dma_start(d_dbg[w * 128:(w + 1) * 128, :], dsb[:])
                    d2 = hpool.tile([128, 524], F32, tag="d2sb")
                    nc.vector.tensor_copy(d2[:, 0:260], pm2[:, 0:260])
                    nc.vector.tensor_copy(d2[:, 260:264], er_res[:, w, :])
                    nc.vector.tensor_copy(d2[:, 264:524], s_sb[:])
                    nc.sync.dma_start(d_dbg2[w * 128:(w + 1) * 128, :],
                                      d2[:])
                if l < 2:
                    nc.scalar.activation(h_keep[:, w, :], hn[:], AF.Relu)
                else:
                    osb = hpool.tile([128, cfg.DH], F32, tag="osb")
                    nc.vector.tensor_reduce(
                        osb[:],
                        hn[:].rearrange("p (h d) -> p d h", h=4),
                        mybir.AxisListType.X, mybir.AluOpType.add)
                    nc.vector.tensor_scalar(osb[:], osb[:], 1.0 / cfg.H, None,
                                            mybir.AluOpType.mult)
                    nc.sync.dma_start(d_out[w * 128:(w + 1) * 128, :], osb[:])

    nc.compile()
    return nc


def make_in_maps(cfg, meta, wnp, num_cores):
    w_all, b_rep, rw, has_b = wnp
    D, Dr, ones, eps, padrow = _consts(cfg)
    ident = np.eye(128, dtype=np.float32)
    maps = []
    for c in range(num_cores):
        m = {
            "xT": _bf16(meta["xT"][c]),
            "w_all": _bf16(w_all), "resW0": _bf16(rw), "ident": _bf16(ident),
            "Dmat": D, "Drmat": Dr, "ones1": ones, "epsrow": eps,
            "padrow": padrow,
            "idx16": meta["idx16"][c],
            "st": _bf16(meta["st"][c]),
            "dstrow": _bf16(meta["dstrow"][c]),
        }
        if has_b:
            m["b_rep"] = b_rep.astype(np.float32)
        maps.append(m)
    return maps


def assemble_output(cfg, meta, results):
    out = np.empty((cfg.N, cfg.DH), np.float32)
    full = np.concatenate([r["out"] for r in results], axis=0)
    out[:] = full[meta["perm_pos"]]
    return out


def solve(x, edge_index, Ws, als, ars, bs, resW0, cfg, trace=False):
    meta = preprocess(x, edge_index, cfg)
    wnp = pack_weights(cfg, Ws, als, ars, bs, resW0)
    nc = build_program(cfg, meta, cfg.NCORES, has_b=wnp[3])
    in_maps = make_in_maps(cfg, meta, wnp, cfg.NCORES)
    res = bass_utils.run_bass_kernel_spmd(
        nc, in_maps, core_ids=list(range(cfg.NCORES)), trace=trace)
    out = assemble_output(cfg, meta, res.results)
    return out, res


def kernel(x, edge_index, W0, W1, W2, al0, al1, al2, ar0, ar1, ar2,
           b0, b1, b2, resW0):
    cfg = Cfg(n=50000, e=800000, fin=128, h=4, dh=64, ncores=8, wpc=49)
    out, _ = solve(np.asarray(x, np.float32), np.asarray(edge_index),
                   [np.asarray(W0, np.float32), np.asarray(W1, np.float32),
                    np.asarray(W2, np.float32)],
                   [np.asarray(al0, np.float32), np.asarray(al1, np.float32),
                    np.asarray(al2, np.float32)],
                   [np.asarray(ar0, np.float32), np.asarray(ar1, np.float32),
                    np.asarray(ar2, np.float32)],
                   [np.asarray(b0, np.float32), np.asarray(b1, np.float32),
                    np.asarray(b2, np.float32)],
                   np.asarray(resW0, np.float32), cfg)
    return out


# revision 41
# speedup vs baseline: 1.7594x; 1.3140x over previous
"""3-layer GAT (DGL GATConv semantics) on 8 Trainium2 NeuronCores.

v2 strategy (graph-parallel; see sim_v2.py for the numpy golden model):
  - Host load-balances dst nodes into 8*49 windows of 128 dsts (LPT packing);
    node order is permuted accordingly, output inverse-permuted at the end.
  - Per layer: each core projects its local nodes ([feat|el|er] in one bf16
    matmul), packs [feat bf16 | el f32] into a 768B G-row plus one dedicated
    pad row per core (feat=0, el=-60 so padded edges vanish in exp), and
    AllGathers G.
  - Edge phase per 128-dst window: dma_gather source G-rows (edges presorted
    by dst; 4 SWDGE queues round-robin so descriptor emission runs on all
    four Q7 core pairs), build ONE is_ge staircase mask B per tile, and use
    it twice with suffix-sum telescoping:
      * er per edge = B.T @ diff(er) (first-difference along dst rows,
        bf16 hi/lo split for accuracy),
      * per-dst segments = first-difference (bidiagonal matmul) of the
        suffix sums B.T-accumulated over tiles, with the softmax denominator
        fused as 4 extra matmul columns and the 1e-16 clamp as a rank-1
        epsilon matmul.
    LeakyReLU/exp/copies run on the Scalar engine; the Vector engine only
    does the mask build, logit add, weighted-feature multiply, and the
    normalization arithmetic. h stays resident in SBUF (bf16) across layers
    (it is both the next layer's input and its residual).
"""

from contextlib import ExitStack

import numpy as np

import concourse.bass as bass  # noqa: F401
import concourse.bacc as bacc
import concourse.mybir as mybir
import concourse.tile as tile
from concourse import bass_utils

F32 = mybir.dt.float32
BF16 = mybir.dt.bfloat16
I16 = mybir.dt.int16

GROW = 384  # bf16 elements per G row: [feat 256 | el-as-f32 8 | pad] = 768B
GMAX = 1024


def _bf16(a):
    import ml_dtypes
    return np.asarray(a, np.float32).astype(ml_dtypes.bfloat16)


class Cfg:
    def __init__(self, n, e, fin, h, dh, ncores, wpc, lo_rows=32768):
        self.N, self.E, self.FIN, self.H, self.DH = n, e, fin, h, dh
        self.HID = h * dh
        self.NCORES, self.WPC = ncores, wpc
        self.NPC = wpc * 128            # nodes per core (excl. pad row)
        self.NPC1 = self.NPC + 1        # + dedicated pad G-row
        self.NPAD = ncores * self.NPC   # padded global nodes
        self.NPAD8 = ncores * self.NPC1  # G-table rows incl. pad rows
        assert self.NPAD >= n
        self.LO_ROWS = min(lo_rows, self.NPAD8)
        self.HI_OFF = max(self.NPAD8 - self.LO_ROWS, 0)
        self.KT = self.HID // 128       # K tiles for layers 1,2
        self.KT0 = fin // 128           # K tiles for layer 0
        self.PAD_LO = self.NPC          # core 0's pad row (< LO_ROWS)
        self.PAD_HI = self.NPAD8 - 1 - self.HI_OFF  # core 7's pad row - HI_OFF


def _lpt_windows(deg, cfg):
    """Assign nodes to ncores*wpc windows of exactly 128 slots, balancing
    per-window edge counts (LPT greedy). Returns perm_pos[node] -> slot."""
    import heapq

    nw = cfg.NCORES * cfg.WPC
    order = np.argsort(-deg, kind="stable")
    heap = [(0, w) for w in range(nw)]
    heapq.heapify(heap)
    counts = np.zeros(nw, np.int64)
    wsum = np.zeros(nw, np.int64)
    assign = np.empty(cfg.N, np.int64)
    for n in order:
        while True:
            s, w = heapq.heappop(heap)
            if counts[w] < 128:
                break
        assign[n] = w
        counts[w] += 1
        wsum[w] += deg[n]
        if counts[w] < 128:
            heapq.heappush(heap, (wsum[w], w))
    perm_pos = np.empty(cfg.N, np.int64)
    next_row = np.zeros(nw, np.int64)
    for n in range(cfg.N):
        w = assign[n]
        perm_pos[n] = w * 128 + next_row[w]
        next_row[w] += 1
    return perm_pos


def _wrap16(vals, cap):
    """Wrap a list of idx values into the Q7 [16, cap//16] layout."""
    out = np.zeros((16, cap // 16), np.int16)
    j = np.arange(len(vals))
    out[j % 16, j // 16] = vals.astype(np.int16)
    return out


def preprocess(x, edge_index, cfg):
    src = np.asarray(edge_index[0], dtype=np.int64)
    dst = np.asarray(edge_index[1], dtype=np.int64)
    deg = np.bincount(dst, minlength=cfg.N)
    perm_pos = _lpt_windows(deg, cfg)

    psrc = perm_pos[src]
    pdst = perm_pos[dst]
    g_of = (psrc // cfg.NPC) * cfg.NPC1 + (psrc % cfg.NPC)  # G-table rows
    gw = pdst // 128
    row = pdst % 128

    ncores, wpc = cfg.NCORES, cfg.WPC
    core = gw // wpc
    wi = gw % wpc
    is_lo = g_of < cfg.LO_ROWS

    ek = (core * wpc + wi) * 2 + (~is_lo).astype(np.int64)
    sort_idx = np.lexsort((row, ek))
    s_g = g_of[sort_idx]
    s_row = row[sort_idx]
    s_ek = ek[sort_idx]
    grp_start = np.searchsorted(s_ek, np.arange(ncores * wpc * 2 + 1))

    lo_t = np.zeros(wpc, np.int64)
    hi_t = np.zeros(wpc, np.int64)
    for w in range(wpc):
        for c in range(ncores):
            g = (c * wpc + w) * 2
            nlo = grp_start[g + 1] - grp_start[g]
            nhi = grp_start[g + 2] - grp_start[g + 1]
            lo_t[w] = max(lo_t[w], (nlo + 127) // 128)
            hi_t[w] = max(hi_t[w], (nhi + 127) // 128)
    T = lo_t + hi_t
    TT = int(T.sum())
    toff = np.concatenate([[0], np.cumsum(T)]).astype(np.int64)
    nidx = TT * 8

    idx16 = np.zeros((ncores, 16, nidx), np.int16)
    # st defaults to 128 so tiles a window reserves beyond this core's own
    # group tile count get an all-zero mask (B = e >= 128 never true).
    st = np.full((ncores, 128, TT), 128.0, np.float32)
    # dstrow defaults to -1 so unwritten tiles get an all-zero edge mask
    # (d <= dstrow never true); pad edge slots also use -1.
    dstrow = np.full((ncores, 128, TT), -1.0, np.float32)

    for c in range(ncores):
        for w in range(wpc):
            base_t = toff[w]
            for reg in (0, 1):
                g = (c * wpc + w) * 2 + reg
                lo, hi_ = grp_start[g], grp_start[g + 1]
                e_g = s_g[lo:hi_]
                e_r = s_row[lo:hi_]
                nt = lo_t[w] if reg == 0 else hi_t[w]
                if nt == 0:
                    assert len(e_g) == 0
                    continue
                cap = nt * 128
                n_e = len(e_g)
                pad = cfg.PAD_LO if reg == 0 else cfg.PAD_HI
                vals = np.full(cap, pad, np.int64)
                vals[:n_e] = e_g if reg == 0 else e_g - cfg.HI_OFF
                assert (vals >= 0).all() and (vals < cfg.LO_ROWS).all()
                rt0 = base_t + (0 if reg == 0 else lo_t[w])
                idx16[c, :, rt0 * 8:(rt0 + nt) * 8] = _wrap16(vals, cap)
                rows = np.full(cap, -1, np.int64)
                rows[:n_e] = e_r
                dstrow[c, :, rt0:rt0 + nt] = rows.reshape(nt, 128).T
                seg = np.searchsorted(e_r, np.arange(129))
                for tl in range(nt):
                    b = tl * 128
                    st[c, :, rt0 + tl] = np.clip(seg[:-1] - b, 0, 128)

    xT = np.zeros((ncores, cfg.FIN, cfg.NPC), np.float32)
    inv_rows = np.full(cfg.NPAD, -1, np.int64)
    inv_rows[perm_pos] = np.arange(cfg.N)
    xf = np.asarray(x, np.float32)
    for c in range(ncores):
        sl = inv_rows[c * cfg.NPC:(c + 1) * cfg.NPC]
        valid = sl >= 0
        xc = np.zeros((cfg.NPC, cfg.FIN), np.float32)
        xc[valid] = xf[sl[valid]]
        xT[c] = xc.T

    # HW Q7 cores each read their own 16-partition group: replicate.
    idx16 = np.tile(idx16, (1, 8, 1))

    # host-precomputed masks (layer-invariant graph structure):
    #  bm[d, t, e]  = (e >= st[d, t])      dst-partitioned staircase
    #  bme[e, t, d] = (d <= dstrow[e, t])  edge-partitioned dual
    import ml_dtypes
    ar = np.arange(128, dtype=np.float32)
    bm = (ar[None, None, None, :] >= st[:, :, :, None]
          ).astype(ml_dtypes.bfloat16)
    bme = (ar[None, None, None, :] <= dstrow[:, :, :, None]
           ).astype(ml_dtypes.bfloat16)

    return dict(perm_pos=perm_pos, lo_t=lo_t, hi_t=hi_t, T=T, TT=TT,
                toff=toff, nidx=nidx, idx16=idx16, bm=bm, bme=bme, xT=xT)


def pack_weights(cfg, Ws, als, ars, bs, resW0):
    # d-major feature layout on device: feature column (h*DH + d) lives at
    # position (d*H + h), so the per-head attention broadcasts are inner
    # stride-1 on the DVE (2x mode).
    H, DH, HID = cfg.H, cfg.DH, cfg.HID
    perm = np.empty(HID, np.int64)  # perm[new] = old
    for d in range(DH):
        for h in range(H):
            perm[d * H + h] = h * DH + d
    kts = []
    for l, W in enumerate(Ws):
        ALf = np.zeros((HID, H), np.float32)
        ARf = np.zeros((HID, H), np.float32)
        for h in range(H):
            ALf[h * DH:(h + 1) * DH, h] = als[l][h]
            ARf[h * DH:(h + 1) * DH, h] = ars[l][h]
        W = np.asarray(W, np.float32)
        if l > 0:
            # input h arrives d-major: permute contraction rows only
            # (W's columns — the feat axis ALf/ARf contract — stay original)
            W = W[perm, :]
        Wc = np.concatenate([W[:, perm], W @ ALf, W @ ARf], axis=1)
        fin = W.shape[0]
        for k in range(fin // 128):
            kts.append(Wc[k * 128:(k + 1) * 128])
    w_all = np.transpose(np.stack(kts), (1, 0, 2)).copy()  # [128, nk, 264]
    b_rep = np.stack([np.tile(b[None, perm], (128, 1)) for b in bs], axis=1)
    has_b = any(np.abs(b).max() > 0 for b in bs)
    rw = np.asarray(resW0, np.float32)[:, perm]
    return w_all, b_rep, rw, has_b


def _consts(cfg):
    # D: pm2[d] = pm[d] - pm[d+1]  (lhsT[k, d]: +1 at k=d, -1 at k=d+1)
    D = np.zeros((128, 128), np.float32)
    Dr = np.zeros((128, 128), np.float32)
    for d in range(128):
        D[d, d] = 1.0
        if d + 1 < 128:
            D[d + 1, d] = -1.0
        # Dr: er_diff[d] = er[d] - er[d-1]
        Dr[d, d] = 1.0
        if d - 1 >= 0:
            Dr[d - 1, d] = -1.0
    ones = np.ones((1, 128), np.float32)
    eps = np.zeros((1, 264), np.float32)
    eps[0, 256:260] = 1e-16
    # pad G-row: feat 0, el = -60.0 (4x f32) at bf16 cols 256:264
    pad = np.zeros(384, np.uint16)
    el = np.full(4, -60.0, np.float32)
    pad[256:264] = el.view(np.uint16)
    import ml_dtypes
    padrow = pad.view(ml_dtypes.bfloat16).reshape(1, 384)
    return D, Dr, ones, eps, padrow


def _gather(nc, out_ap, in_ap, idx_sb, col0, n, elem, q=0):
    """One dma_gather call for n indices. q selects the SWDGE queue: queue
    q's descriptors are emitted by Q7 core pair (2q, 2q+1), so spreading
    windows across queues parallelizes descriptor generation 4x."""
    nc.gpsimd.dma_gather(
        out_ap=out_ap, in_ap=in_ap,
        idxs_ap=idx_sb[:, col0: col0 + n // 16],
        num_idxs=n, num_idxs_reg=n, elem_size=elem,
        single_packet=(n <= GMAX), queue_num=q)


def build_program(cfg, meta, num_cores, has_b=False, dbg_layer=None):
    nc = bacc.Bacc("TRN2", target_bir_lowering=False, debug=False,
                   num_devices=num_cores, num_swdge_queues=4)
    NPC, HID = cfg.NPC, cfg.HID
    WPC = cfg.WPC
    NK = cfg.KT0 + 2 * cfg.KT
    lo_t, hi_t, T, toff, TT = meta["lo_t"], meta["hi_t"], meta["T"], \
        meta["toff"], meta["TT"]
    maxT = int(T.max())
    AF = mybir.ActivationFunctionType

    d_xT = nc.dram_tensor("xT", [cfg.FIN, NPC], BF16, kind="ExternalInput")
    d_w = nc.dram_tensor("w_all", [128, NK, 264], BF16, kind="ExternalInput")
    d_rw = nc.dram_tensor("resW0", [128, 256], BF16, kind="ExternalInput")
    d_id = nc.dram_tensor("ident", [128, 128], BF16, kind="ExternalInput")
    d_D = nc.dram_tensor("Dmat", [128, 128], F32, kind="ExternalInput")
    d_Dr = nc.dram_tensor("Drmat", [128, 128], F32, kind="ExternalInput")
    d_ones = nc.dram_tensor("ones1", [1, 128], F32, kind="ExternalInput")
    d_eps = nc.dram_tensor("epsrow", [1, 264], F32, kind="ExternalInput")
    d_pad = nc.dram_tensor("padrow", [1, GROW], BF16, kind="ExternalInput")
    d_idx = nc.dram_tensor("idx16", [128, meta["nidx"]], I16,
                           kind="ExternalInput")
    d_bm = nc.dram_tensor("bm", [128, TT, 128], BF16, kind="ExternalInput")
    d_bme = nc.dram_tensor("bme", [128, TT, 128], BF16, kind="ExternalInput")
    d_out = nc.dram_tensor("out", [NPC, cfg.DH], F32, kind="ExternalOutput")
    if has_b:
        d_b = nc.dram_tensor("b_rep", [128, 3, 256], F32,
                             kind="ExternalInput")
    if dbg_layer is not None:
        d_dbg = nc.dram_tensor("dbg", [NPC, HID], F32, kind="ExternalOutput")
        d_dbg2 = nc.dram_tensor("dbg2", [NPC, 524], F32,
                                kind="ExternalOutput")
        d_dbg3 = nc.dram_tensor("dbg3", [128, maxT * 128], F32,
                                kind="ExternalOutput")
        d_dbg4 = nc.dram_tensor("dbg4", [128, maxT * 8], F32,
                                kind="ExternalOutput")

    with ExitStack() as ctx:
        tc = ctx.enter_context(tile.TileContext(nc))
        cpool = ctx.enter_context(tc.tile_pool(name="const", bufs=1))
        dram = ctx.enter_context(tc.tile_pool(name="dram", bufs=1,
                                              space="DRAM"))
        fgpool = ctx.enter_context(tc.tile_pool(name="fg", bufs=4))
        bpool = ctx.enter_context(tc.tile_pool(name="bm", bufs=3))
        epool = ctx.enter_context(tc.tile_pool(name="e", bufs=3))
        wpool = ctx.enter_context(tc.tile_pool(name="wt", bufs=3))
        hpool = ctx.enter_context(tc.tile_pool(name="h", bufs=4))
        gpool = ctx.enter_context(tc.tile_pool(name="g", bufs=3))
        spool = ctx.enter_context(tc.tile_pool(name="s", bufs=3))
        ps_m = ctx.enter_context(tc.tile_pool(name="psm", bufs=2,
                                              space="PSUM"))
        ps_m2 = ctx.enter_context(tc.tile_pool(name="psm2", bufs=1,
                                               space="PSUM"))
        ps_p = ctx.enter_context(tc.tile_pool(name="psp", bufs=2,
                                              space="PSUM"))
        ps_t = ctx.enter_context(tc.tile_pool(name="pst", bufs=1,
                                              space="PSUM"))
        ps_e = ctx.enter_context(tc.tile_pool(name="pse", bufs=1,
                                              space="PSUM"))
        ps_q = ctx.enter_context(tc.tile_pool(name="psq", bufs=1,
                                              space="PSUM"))

        g_loc = dram.tile([cfg.NPC1, GROW], BF16)
        g_fulls = [
            dram.tile([cfg.NPAD8, GROW], BF16, name=f"g_full{i}",
                      addr_space="Shared" if num_cores > 4 else "Local")
            for i in range(3)]

        # ---- persistent SBUF state ----
        xT_sb = cpool.tile([128, NPC], BF16)
        nc.sync.dma_start(xT_sb[:], d_xT[:])
        w_sb = cpool.tile([128, NK, 264], BF16)
        nc.sync.dma_start(w_sb[:], d_w[:])
        rw_sb = cpool.tile([128, 256], BF16)
        nc.sync.dma_start(rw_sb[:], d_rw[:])
        id_sb = cpool.tile([128, 128], BF16)
        nc.sync.dma_start(id_sb[:], d_id[:])
        D_sb = cpool.tile([128, 128], F32)
        nc.sync.dma_start(D_sb[:], d_D[:])
        Dr_sb = cpool.tile([128, 128], F32)
        nc.sync.dma_start(Dr_sb[:], d_Dr[:])
        ones_sb = cpool.tile([1, 128], F32)
        nc.sync.dma_start(ones_sb[:], d_ones[:])
        eps_sb = cpool.tile([1, 264], F32)
        nc.sync.dma_start(eps_sb[:], d_eps[:])
        pad_sb = cpool.tile([1, GROW], BF16)
        nc.sync.dma_start(pad_sb[:], d_pad[:])
        idx_sb = cpool.tile([128, meta["nidx"]], I16)
        nc.sync.dma_start(idx_sb[:], d_idx[:])

        if has_b:
            b_sb = cpool.tile([128, 3, 256], F32)
            nc.sync.dma_start(b_sb[:], d_b[:])
        er_res = cpool.tile([128, WPC, 4], F32)
        h_keep = cpool.tile([128, WPC, HID], BF16)

        kt_of_layer = [list(range(cfg.KT0)),
                       list(range(cfg.KT0, cfg.KT0 + cfg.KT)),
                       list(range(cfg.KT0 + cfg.KT, NK))]

        for l in range(3):
            # ---------------- projection phase ----------------
            for nt in range(WPC):
                kts = kt_of_layer[l]
                lhsTs = []
                if l == 0:
                    lhsTs.append(xT_sb[:, nt * 128:(nt + 1) * 128])
                else:
                    for ft in range(cfg.KT):
                        pst = ps_t.tile([128, 128], BF16, tag="pt")
                        nc.tensor.transpose(
                            pst[:],
                            h_keep[:, nt, ft * 128:(ft + 1) * 128],
                            id_sb[:])
                        hT = hpool.tile([128, 128], BF16, tag="lhsT")
                        nc.scalar.copy(hT[:], pst[:])
                        lhsTs.append(hT[:])
                pp = ps_p.tile([128, 264], F32, tag="pp")
                for k, (kt, lt) in enumerate(zip(kts, lhsTs)):
                    nc.tensor.matmul(pp[:], lt, w_sb[:, kt, :],
                                     start=(k == 0), stop=(k == len(kts) - 1))
                g_sb = gpool.tile([128, 264], BF16)
                nc.scalar.copy(g_sb[:, 0:256], pp[:, 0:256])
                nc.scalar.copy(g_sb[:, 256:264].bitcast(F32), pp[:, 256:260])
                nc.scalar.copy(er_res[:, nt, :], pp[:, 260:264])
                nc.sync.dma_start(g_loc[nt * 128:(nt + 1) * 128, 0:264],
                                  g_sb[:, 0:264])
                if l == 0:
                    pr = ps_p.tile([128, 256], F32, tag="pp")
                    nc.tensor.matmul(pr[:], lhsTs[0], rw_sb[:],
                                     start=True, stop=True)
                    # layer-0 residual (x @ resW0) parked in h_keep
                    nc.scalar.copy(h_keep[:, nt, :], pr[:])
            nc.sync.dma_start(g_loc[NPC:NPC + 1, :], pad_sb[:])

            g_full = g_fulls[l]
            nc.gpsimd.collective_compute(
                "AllGather", mybir.AluOpType.bypass,
                replica_groups=[list(range(num_cores))],
                ins=[g_loc.opt()], outs=[g_full.opt()])

            # ---------------- aggregation phase ----------------
            for w in range(WPC):
                Tw = int(T[w])
                lt_, ht_ = int(lo_t[w]), int(hi_t[w])
                t0 = int(toff[w])

                # er first-difference with bf16 hi/lo split
                er_dq = ps_q.tile([128, 4], F32, tag="edq")
                nc.tensor.matmul(er_dq[:], Dr_sb[:], er_res[:, w, :],
                                 start=True, stop=True)
                er8 = epool.tile([128, 8], BF16, tag="er8")
                nc.scalar.copy(er8[:, 0:4], er_dq[:])
                nc.vector.tensor_tensor(er8[:, 4:8], er_dq[:], er8[:, 0:4],
                                        mybir.AluOpType.subtract)

                fg = fgpool.tile([128, maxT, GROW], BF16)
                if lt_ > 0:
                    _gather(nc, fg[:, 0:lt_, :], g_full[0:cfg.LO_ROWS, :],
                            idx_sb, t0 * 8, lt_ * 128, GROW, q=w % 4)
                if ht_ > 0:
                    _gather(nc, fg[:, lt_:Tw, :],
                            g_full[cfg.HI_OFF:cfg.NPAD8, :], idx_sb,
                            (t0 + lt_) * 8, ht_ * 128, GROW, q=w % 4)

                # host-precomputed masks (layer-invariant):
                #  bm[d, t, e] = (e >= st[d, t])   for the er matmul
                #  bme[e, t, d] = (d <= dstrow)    for the aggregation matmul
                bm = bpool.tile([128, maxT, 128], BF16, tag="bmd")
                nc.sync.dma_start(bm[:, 0:Tw, :], d_bm[:, t0:t0 + Tw, :])
                bme = bpool.tile([128, maxT, 128], BF16, tag="bme")
                nc.sync.dma_start(bme[:, 0:Tw, :], d_bme[:, t0:t0 + Tw, :])

                # er per edge via telescoping (hi+lo columns)
                pe8 = ps_e.tile([128, maxT, 8], F32, tag="pe8")
                for t in range(Tw):
                    nc.tensor.matmul(pe8[:, t, :], bm[:, t, :], er8[:],
                                     start=True, stop=True)

                # logits + LeakyReLU + exp (exp lands in wsb cols 256:260)
                el_v = fg[:, 0:Tw, 256:264].bitcast(F32)    # [128, Tw, 4]
                e_sb = epool.tile([128, maxT, 4], F32, tag="e")
                nc.vector.tensor_tensor(e_sb[:, 0:Tw, :], el_v,
                                        pe8[:, 0:Tw, 0:4],
                                        mybir.AluOpType.add)
                nc.vector.tensor_tensor(e_sb[:, 0:Tw, :], e_sb[:, 0:Tw, :],
                                        pe8[:, 0:Tw, 4:8],
                                        mybir.AluOpType.add)
                nc.vector.scalar_tensor_tensor(
                    e_sb[:, 0:Tw, :], e_sb[:, 0:Tw, :], 0.2, e_sb[:, 0:Tw, :],
                    mybir.AluOpType.mult, mybir.AluOpType.max)
                exb = epool.tile([128, maxT, 4], BF16, tag="exb")
                nc.scalar.activation(exb[:, 0:Tw, :], e_sb[:, 0:Tw, :],
                                     AF.Exp)
                wsb = wpool.tile([128, maxT, 260], BF16)
                nc.vector.tensor_copy(wsb[:, 0:Tw, 256:260], exb[:, 0:Tw, :])

                # weighted feats (d-major): wsb[e,t,d,h] = fg[e,t,d,h]*ex[e,t,h]
                # inner stride-1 on h for both sources -> DVE 2x mode
                nc.vector.tensor_tensor(
                    wsb[:, 0:Tw, 0:256].rearrange("p t (d h) -> p t d h",
                                                  h=4),
                    fg[:, 0:Tw, 0:256].rearrange("p t (d h) -> p t d h", h=4),
                    exb[:, 0:Tw, :].unsqueeze(2).broadcast_to(
                        [128, Tw, 64, 4]),
                    mybir.AluOpType.mult)

                # suffix sums over tiles: pm[d] = sum_t BT_t.T @ wsb_t
                pm = ps_m.tile([128, 260], F32)
                for t in range(Tw):
                    nc.tensor.matmul(pm[:], bme[:, t, :], wsb[:, t, :],
                                     start=(t == 0), stop=(t == Tw - 1),
                                     skip_group_check=True)

                # telescope to per-dst segments + epsilon clamp
                s_sb = spool.tile([128, 260], F32)
                nc.scalar.copy(s_sb[:], pm[:])
                pm2 = ps_m2.tile([128, 260], F32)
                nc.tensor.matmul(pm2[:], D_sb[:], s_sb[:],
                                 start=True, stop=False)
                nc.tensor.matmul(pm2[:], ones_sb[:], eps_sb[0:1, 0:260],
                                 start=False, stop=True)

                rden = epool.tile([128, 4], F32, tag="rden")
                nc.vector.reciprocal(rden[:], pm2[:, 256:260])
                hn = hpool.tile([128, HID], F32, tag="hn")
                nc.vector.tensor_tensor(
                    hn[:].rearrange("p (d h) -> p d h", h=4),
                    pm2[:, 0:256].rearrange("p (d h) -> p d h", h=4),
                    rden[:].unsqueeze(1).broadcast_to([128, 64, 4]),
                    mybir.AluOpType.mult)
                nc.vector.tensor_tensor(hn[:], hn[:], h_keep[:, w, :],
                                        mybir.AluOpType.add)
                if has_b:
                    nc.vector.tensor_tensor(hn[:], hn[:], b_sb[:, l, :],
                                            mybir.AluOpType.add)
                if dbg_layer is not None and l == dbg_layer and w == 0:
                    d3 = hpool.tile([128, maxT * 128], F32, tag="d3sb")
                    nc.vector.tensor_copy(
                        d3[:, 0:maxT * 4].rearrange("p (t e) -> p t e",
                                                    t=maxT),
                        fg[:, 0:maxT, 256:264].bitcast(F32))
                    nc.vector.tensor_copy(
                        d3[:, maxT * 4:maxT * 4 + maxT * 64]
                        .rearrange("p (t e) -> p t e", t=maxT),
                        fg[:, 0:maxT, 0:64])
                    nc.sync.dma_start(d_dbg3[:], d3[:])
                    d4 = hpool.tile([128, maxT * 8], F32, tag="d4sb")
                    nc.vector.tensor_copy(
                        d4[:].rearrange("p (t e) -> p t e", t=maxT),
                        pe8[:, 0:maxT, :])
                    nc.sync.dma_start(d_dbg4[:], d4[:])
                if dbg_layer is not None and l == dbg_layer:
                    dsb = hpool.tile([128, HID], F32, tag="dsb")
                    nc.vector.tensor_copy(dsb[:], hn[:])
                    nc.sync.dma_start(d_dbg[w * 128:(w + 1) * 128, :], dsb[:])
                    d2 = hpool.tile([128, 524], F32, tag="d2sb")
                    nc.vector.tensor_copy(d2[:, 0:260], pm2[:, 0:260])
                    nc.vector.tensor_copy(d2[:, 260:264], er_res[:, w, :])
                    nc.vector.tensor_copy(d2[:, 264:524], s_sb[:])
                    nc.sync.dma_start(d_dbg2[w * 128:(w + 1) * 128, :],
                                      d2[:])
                if l < 2:
                    nc.scalar.activation(h_keep[:, w, :], hn[:], AF.Relu)
                else:
                    osb = hpool.tile([128, cfg.DH], F32, tag="osb")
                    nc.vector.tensor_reduce(
                        osb[:],
                        hn[:].rearrange("p (d h) -> p d h", h=4),
                        mybir.AxisListType.X, mybir.AluOpType.add)
                    nc.vector.tensor_scalar(osb[:], osb[:], 1.0 / cfg.H, None,
                                            mybir.AluOpType.mult)
                    nc.sync.dma_start(d_out[w * 128:(w + 1) * 128, :], osb[:])

    nc.compile()
    return nc


def make_in_maps(cfg, meta, wnp, num_cores):
    w_all, b_rep, rw, has_b = wnp
    D, Dr, ones, eps, padrow = _consts(cfg)
    ident = np.eye(128, dtype=np.float32)
    maps = []
    for c in range(num_cores):
        m = {
            "xT": _bf16(meta["xT"][c]),
            "w_all": _bf16(w_all), "resW0": _bf16(rw), "ident": _bf16(ident),
            "Dmat": D, "Drmat": Dr, "ones1": ones, "epsrow": eps,
            "padrow": padrow,
            "idx16": meta["idx16"][c],
            "bm": meta["bm"][c],
            "bme": meta["bme"][c],
        }
        if has_b:
            m["b_rep"] = b_rep.astype(np.float32)
        maps.append(m)
    return maps


def assemble_output(cfg, meta, results):
    out = np.empty((cfg.N, cfg.DH), np.float32)
    full = np.concatenate([r["out"] for r in results], axis=0)
    out[:] = full[meta["perm_pos"]]
    return out


def solve(x, edge_index, Ws, als, ars, bs, resW0, cfg, trace=False):
    meta = preprocess(x, edge_index, cfg)
    wnp = pack_weights(cfg, Ws, als, ars, bs, resW0)
    nc = build_program(cfg, meta, cfg.NCORES, has_b=wnp[3])
    in_maps = make_in_maps(cfg, meta, wnp, cfg.NCORES)
    res = bass_utils.run_bass_kernel_spmd(
        nc, in_maps, core_ids=list(range(cfg.NCORES)), trace=trace)
    out = assemble_output(cfg, meta, res.results)
    return out, res


def kernel(x, edge_index, W0, W1, W2, al0, al1, al2, ar0, ar1, ar2,
           b0, b1, b2, resW0):
    cfg = Cfg(n=50000, e=800000, fin=128, h=4, dh=64, ncores=8, wpc=49)
    out, _ = solve(np.asarray(x, np.float32), np.asarray(edge_index),
                   [np.asarray(W0, np.float32), np.asarray(W1, np.float32),
                    np.asarray(W2, np.float32)],
                   [np.asarray(al0, np.float32), np.asarray(al1, np.float32),
                    np.asarray(al2, np.float32)],
                   [np.asarray(ar0, np.float32), np.asarray(ar1, np.float32),
                    np.asarray(ar2, np.float32)],
                   [np.asarray(b0, np.float32), np.asarray(b1, np.float32),
                    np.asarray(b2, np.float32)],
                   np.asarray(resW0, np.float32), cfg)
    return out


# revision 43
# speedup vs baseline: 1.8561x; 1.0550x over previous
"""3-layer GAT (DGL GATConv semantics) on 8 Trainium2 NeuronCores.

v2 strategy (graph-parallel; see sim_v2.py for the numpy golden model):
  - Host load-balances dst nodes into 8*49 windows of 128 dsts (LPT packing);
    node order is permuted accordingly, output inverse-permuted at the end.
  - Per layer: each core projects its local nodes ([feat|el|er] in one bf16
    matmul), packs [feat bf16 | el f32] into a 768B G-row plus one dedicated
    pad row per core (feat=0, el=-60 so padded edges vanish in exp), and
    AllGathers G.
  - Edge phase per 128-dst window: dma_gather source G-rows (edges presorted
    by dst; 4 SWDGE queues round-robin so descriptor emission runs on all
    four Q7 core pairs), build ONE is_ge staircase mask B per tile, and use
    it twice with suffix-sum telescoping:
      * er per edge = B.T @ diff(er) (first-difference along dst rows,
        bf16 hi/lo split for accuracy),
      * per-dst segments = first-difference (bidiagonal matmul) of the
        suffix sums B.T-accumulated over tiles, with the softmax denominator
        fused as 4 extra matmul columns and the 1e-16 clamp as a rank-1
        epsilon matmul.
    LeakyReLU/exp/copies run on the Scalar engine; the Vector engine only
    does the mask build, logit add, weighted-feature multiply, and the
    normalization arithmetic. h stays resident in SBUF (bf16) across layers
    (it is both the next layer's input and its residual).
"""

from contextlib import ExitStack

import numpy as np

import concourse.bass as bass  # noqa: F401
import concourse.bacc as bacc
import concourse.mybir as mybir
import concourse.tile as tile
from concourse import bass_utils

F32 = mybir.dt.float32
BF16 = mybir.dt.bfloat16
I16 = mybir.dt.int16
F8 = mybir.dt.float8e4

GROW = 384  # bf16 elements per G row: [feat 256 | el-as-f32 8 | pad] = 768B
GMAX = 1024


def _bf16(a):
    import ml_dtypes
    return np.asarray(a, np.float32).astype(ml_dtypes.bfloat16)


class Cfg:
    def __init__(self, n, e, fin, h, dh, ncores, wpc, lo_rows=32768):
        self.N, self.E, self.FIN, self.H, self.DH = n, e, fin, h, dh
        self.HID = h * dh
        self.NCORES, self.WPC = ncores, wpc
        self.NPC = wpc * 128            # nodes per core (excl. pad row)
        self.NPC1 = self.NPC + 1        # + dedicated pad G-row
        self.NPAD = ncores * self.NPC   # padded global nodes
        self.NPAD8 = ncores * self.NPC1  # G-table rows incl. pad rows
        assert self.NPAD >= n
        self.LO_ROWS = min(lo_rows, self.NPAD8)
        self.HI_OFF = max(self.NPAD8 - self.LO_ROWS, 0)
        self.KT = self.HID // 128       # K tiles for layers 1,2
        self.KT0 = fin // 128           # K tiles for layer 0
        self.PAD_LO = self.NPC          # core 0's pad row (< LO_ROWS)
        self.PAD_HI = self.NPAD8 - 1 - self.HI_OFF  # core 7's pad row - HI_OFF


def _lpt_windows(deg, cfg):
    """Assign nodes to ncores*wpc windows of exactly 128 slots, balancing
    per-window edge counts (LPT greedy). Returns perm_pos[node] -> slot."""
    import heapq

    nw = cfg.NCORES * cfg.WPC
    order = np.argsort(-deg, kind="stable")
    heap = [(0, w) for w in range(nw)]
    heapq.heapify(heap)
    counts = np.zeros(nw, np.int64)
    wsum = np.zeros(nw, np.int64)
    assign = np.empty(cfg.N, np.int64)
    for n in order:
        while True:
            s, w = heapq.heappop(heap)
            if counts[w] < 128:
                break
        assign[n] = w
        counts[w] += 1
        wsum[w] += deg[n]
        if counts[w] < 128:
            heapq.heappush(heap, (wsum[w], w))
    perm_pos = np.empty(cfg.N, np.int64)
    next_row = np.zeros(nw, np.int64)
    for n in range(cfg.N):
        w = assign[n]
        perm_pos[n] = w * 128 + next_row[w]
        next_row[w] += 1
    return perm_pos


def _wrap16(vals, cap):
    """Wrap a list of idx values into the Q7 [16, cap//16] layout."""
    out = np.zeros((16, cap // 16), np.int16)
    j = np.arange(len(vals))
    out[j % 16, j // 16] = vals.astype(np.int16)
    return out


def preprocess(x, edge_index, cfg):
    src = np.asarray(edge_index[0], dtype=np.int64)
    dst = np.asarray(edge_index[1], dtype=np.int64)
    deg = np.bincount(dst, minlength=cfg.N)
    perm_pos = _lpt_windows(deg, cfg)

    psrc = perm_pos[src]
    pdst = perm_pos[dst]
    g_of = (psrc // cfg.NPC) * cfg.NPC1 + (psrc % cfg.NPC)  # G-table rows
    gw = pdst // 128
    row = pdst % 128

    ncores, wpc = cfg.NCORES, cfg.WPC
    core = gw // wpc
    wi = gw % wpc
    is_lo = g_of < cfg.LO_ROWS

    ek = (core * wpc + wi) * 2 + (~is_lo).astype(np.int64)
    sort_idx = np.lexsort((row, ek))
    s_g = g_of[sort_idx]
    s_row = row[sort_idx]
    s_ek = ek[sort_idx]
    grp_start = np.searchsorted(s_ek, np.arange(ncores * wpc * 2 + 1))

    lo_t = np.zeros(wpc, np.int64)
    hi_t = np.zeros(wpc, np.int64)
    for w in range(wpc):
        for c in range(ncores):
            g = (c * wpc + w) * 2
            nlo = grp_start[g + 1] - grp_start[g]
            nhi = grp_start[g + 2] - grp_start[g + 1]
            lo_t[w] = max(lo_t[w], (nlo + 127) // 128)
            hi_t[w] = max(hi_t[w], (nhi + 127) // 128)
    T = lo_t + hi_t
    TT = int(T.sum())
    toff = np.concatenate([[0], np.cumsum(T)]).astype(np.int64)
    nidx = TT * 8

    idx16 = np.zeros((ncores, 16, nidx), np.int16)
    # st defaults to 128 so tiles a window reserves beyond this core's own
    # group tile count get an all-zero mask (B = e >= 128 never true).
    st = np.full((ncores, 128, TT), 128.0, np.float32)
    # dstrow defaults to -1 so unwritten tiles get an all-zero edge mask
    # (d <= dstrow never true); pad edge slots also use -1.
    dstrow = np.full((ncores, 128, TT), -1.0, np.float32)

    for c in range(ncores):
        for w in range(wpc):
            base_t = toff[w]
            for reg in (0, 1):
                g = (c * wpc + w) * 2 + reg
                lo, hi_ = grp_start[g], grp_start[g + 1]
                e_g = s_g[lo:hi_]
                e_r = s_row[lo:hi_]
                nt = lo_t[w] if reg == 0 else hi_t[w]
                if nt == 0:
                    assert len(e_g) == 0
                    continue
                cap = nt * 128
                n_e = len(e_g)
                pad = cfg.PAD_LO if reg == 0 else cfg.PAD_HI
                vals = np.full(cap, pad, np.int64)
                vals[:n_e] = e_g if reg == 0 else e_g - cfg.HI_OFF
                assert (vals >= 0).all() and (vals < cfg.LO_ROWS).all()
                rt0 = base_t + (0 if reg == 0 else lo_t[w])
                idx16[c, :, rt0 * 8:(rt0 + nt) * 8] = _wrap16(vals, cap)
                rows = np.full(cap, -1, np.int64)
                rows[:n_e] = e_r
                dstrow[c, :, rt0:rt0 + nt] = rows.reshape(nt, 128).T
                seg = np.searchsorted(e_r, np.arange(129))
                for tl in range(nt):
                    b = tl * 128
                    st[c, :, rt0 + tl] = np.clip(seg[:-1] - b, 0, 128)

    xT = np.zeros((ncores, cfg.FIN, cfg.NPC), np.float32)
    inv_rows = np.full(cfg.NPAD, -1, np.int64)
    inv_rows[perm_pos] = np.arange(cfg.N)
    xf = np.asarray(x, np.float32)
    for c in range(ncores):
        sl = inv_rows[c * cfg.NPC:(c + 1) * cfg.NPC]
        valid = sl >= 0
        xc = np.zeros((cfg.NPC, cfg.FIN), np.float32)
        xc[valid] = xf[sl[valid]]
        xT[c] = xc.T

    # HW Q7 cores each read their own 16-partition group: replicate.
    idx16 = np.tile(idx16, (1, 8, 1))

    # host-precomputed masks (layer-invariant graph structure):
    #  bm[d, t, e]  = (e >= st[d, t])      dst-partitioned staircase
    #  bme[e, t, d] = (d <= dstrow[e, t])  edge-partitioned dual
    import ml_dtypes
    ar = np.arange(128, dtype=np.float32)
    bm = (ar[None, None, None, :] >= st[:, :, :, None]
          ).astype(ml_dtypes.float8_e4m3fn)
    bme = (ar[None, None, None, :] <= dstrow[:, :, :, None]
           ).astype(ml_dtypes.float8_e4m3fn)

    return dict(perm_pos=perm_pos, lo_t=lo_t, hi_t=hi_t, T=T, TT=TT,
                toff=toff, nidx=nidx, idx16=idx16, bm=bm, bme=bme, xT=xT)


def pack_weights(cfg, Ws, als, ars, bs, resW0):
    # d-major feature layout on device: feature column (h*DH + d) lives at
    # position (d*H + h), so the per-head attention broadcasts are inner
    # stride-1 on the DVE (2x mode).
    H, DH, HID = cfg.H, cfg.DH, cfg.HID
    perm = np.empty(HID, np.int64)  # perm[new] = old
    for d in range(DH):
        for h in range(H):
            perm[d * H + h] = h * DH + d
    kts = []
    for l, W in enumerate(Ws):
        ALf = np.zeros((HID, H), np.float32)
        ARf = np.zeros((HID, H), np.float32)
        for h in range(H):
            ALf[h * DH:(h + 1) * DH, h] = als[l][h]
            ARf[h * DH:(h + 1) * DH, h] = ars[l][h]
        W = np.asarray(W, np.float32)
        if l > 0:
            # input h arrives d-major: permute contraction rows only
            # (W's columns — the feat axis ALf/ARf contract — stay original)
            W = W[perm, :]
        Wc = np.concatenate([W[:, perm], W @ ALf, W @ ARf], axis=1)
        fin = W.shape[0]
        for k in range(fin // 128):
            kts.append(Wc[k * 128:(k + 1) * 128])
    w_all = np.transpose(np.stack(kts), (1, 0, 2)).copy()  # [128, nk, 264]
    b_rep = np.stack([np.tile(b[None, perm], (128, 1)) for b in bs], axis=1)
    has_b = any(np.abs(b).max() > 0 for b in bs)
    rw = np.asarray(resW0, np.float32)[:, perm]
    return w_all, b_rep, rw, has_b


def _consts(cfg):
    # D: pm2[d] = pm[d] - pm[d+1]  (lhsT[k, d]: +1 at k=d, -1 at k=d+1)
    D = np.zeros((128, 128), np.float32)
    Dr = np.zeros((128, 128), np.float32)
    for d in range(128):
        D[d, d] = 1.0
        if d + 1 < 128:
            D[d + 1, d] = -1.0
        # Dr: er_diff[d] = er[d] - er[d-1]
        Dr[d, d] = 1.0
        if d - 1 >= 0:
            Dr[d - 1, d] = -1.0
    ones = np.ones((1, 128), np.float32)
    eps = np.zeros((1, 264), np.float32)
    eps[0, 256:260] = 1e-16
    # pad G-row: feat 0, el = -60.0 (4x f32) at bf16 cols 256:264
    pad = np.zeros(384, np.uint16)
    el = np.full(4, -60.0, np.float32)
    pad[256:264] = el.view(np.uint16)
    import ml_dtypes
    padrow = pad.view(ml_dtypes.bfloat16).reshape(1, 384)
    return D, Dr, ones, eps, padrow


def _gather(nc, out_ap, in_ap, idx_sb, col0, n, elem, q=0):
    """One dma_gather call for n indices. q selects the SWDGE queue: queue
    q's descriptors are emitted by Q7 core pair (2q, 2q+1), so spreading
    windows across queues parallelizes descriptor generation 4x."""
    nc.gpsimd.dma_gather(
        out_ap=out_ap, in_ap=in_ap,
        idxs_ap=idx_sb[:, col0: col0 + n // 16],
        num_idxs=n, num_idxs_reg=n, elem_size=elem,
        single_packet=(n <= GMAX), queue_num=q)


def build_program(cfg, meta, num_cores, has_b=False, dbg_layer=None):
    nc = bacc.Bacc("TRN2", target_bir_lowering=False, debug=False,
                   num_devices=num_cores, num_swdge_queues=4)
    NPC, HID = cfg.NPC, cfg.HID
    WPC = cfg.WPC
    NK = cfg.KT0 + 2 * cfg.KT
    lo_t, hi_t, T, toff, TT = meta["lo_t"], meta["hi_t"], meta["T"], \
        meta["toff"], meta["TT"]
    maxT = int(T.max())
    AF = mybir.ActivationFunctionType

    d_xT = nc.dram_tensor("xT", [cfg.FIN, NPC], BF16, kind="ExternalInput")
    d_w = nc.dram_tensor("w_all", [128, NK, 264], BF16, kind="ExternalInput")
    d_rw = nc.dram_tensor("resW0", [128, 256], BF16, kind="ExternalInput")
    d_id = nc.dram_tensor("ident", [128, 128], BF16, kind="ExternalInput")
    d_D = nc.dram_tensor("Dmat", [128, 128], F32, kind="ExternalInput")
    d_Dr = nc.dram_tensor("Drmat", [128, 128], F32, kind="ExternalInput")
    d_ones = nc.dram_tensor("ones1", [1, 128], F32, kind="ExternalInput")
    d_eps = nc.dram_tensor("epsrow", [1, 264], F32, kind="ExternalInput")
    d_pad = nc.dram_tensor("padrow", [1, GROW], BF16, kind="ExternalInput")
    d_idx = nc.dram_tensor("idx16", [128, meta["nidx"]], I16,
                           kind="ExternalInput")
    d_bm = nc.dram_tensor("bm", [128, TT, 128], F8, kind="ExternalInput")
    d_bme = nc.dram_tensor("bme", [128, TT, 128], F8, kind="ExternalInput")
    d_out = nc.dram_tensor("out", [NPC, cfg.DH], F32, kind="ExternalOutput")
    if has_b:
        d_b = nc.dram_tensor("b_rep", [128, 3, 256], F32,
                             kind="ExternalInput")
    if dbg_layer is not None:
        d_dbg = nc.dram_tensor("dbg", [NPC, HID], F32, kind="ExternalOutput")
        d_dbg2 = nc.dram_tensor("dbg2", [NPC, 524], F32,
                                kind="ExternalOutput")
        d_dbg3 = nc.dram_tensor("dbg3", [128, maxT * 128], F32,
                                kind="ExternalOutput")
        d_dbg4 = nc.dram_tensor("dbg4", [128, maxT * 8], F32,
                                kind="ExternalOutput")

    with ExitStack() as ctx:
        tc = ctx.enter_context(tile.TileContext(nc))
        cpool = ctx.enter_context(tc.tile_pool(name="const", bufs=1))
        dram = ctx.enter_context(tc.tile_pool(name="dram", bufs=1,
                                              space="DRAM"))
        fgpool = ctx.enter_context(tc.tile_pool(name="fg", bufs=5))
        bpool = ctx.enter_context(tc.tile_pool(name="bm", bufs=3))
        epool = ctx.enter_context(tc.tile_pool(name="e", bufs=3))
        wpool = ctx.enter_context(tc.tile_pool(name="wt", bufs=3))
        hpool = ctx.enter_context(tc.tile_pool(name="h", bufs=4))
        gpool = ctx.enter_context(tc.tile_pool(name="g", bufs=3))
        spool = ctx.enter_context(tc.tile_pool(name="s", bufs=3))
        ps_m = ctx.enter_context(tc.tile_pool(name="psm", bufs=2,
                                              space="PSUM"))
        ps_m2 = ctx.enter_context(tc.tile_pool(name="psm2", bufs=2,
                                               space="PSUM"))
        ps_p = ctx.enter_context(tc.tile_pool(name="psp", bufs=1,
                                              space="PSUM"))
        ps_t = ctx.enter_context(tc.tile_pool(name="pst", bufs=1,
                                              space="PSUM"))
        ps_e = ctx.enter_context(tc.tile_pool(name="pse", bufs=1,
                                              space="PSUM"))
        ps_q = ctx.enter_context(tc.tile_pool(name="psq", bufs=1,
                                              space="PSUM"))

        g_loc = dram.tile([cfg.NPC1, GROW], BF16)
        g_fulls = [
            dram.tile([cfg.NPAD8, GROW], BF16, name=f"g_full{i}",
                      addr_space="Shared" if num_cores > 4 else "Local")
            for i in range(3)]

        # ---- persistent SBUF state ----
        xT_sb = cpool.tile([128, NPC], BF16)
        nc.sync.dma_start(xT_sb[:], d_xT[:])
        w_sb = cpool.tile([128, NK, 264], BF16)
        nc.sync.dma_start(w_sb[:], d_w[:])
        rw_sb = cpool.tile([128, 256], BF16)
        nc.sync.dma_start(rw_sb[:], d_rw[:])
        id_sb = cpool.tile([128, 128], BF16)
        nc.sync.dma_start(id_sb[:], d_id[:])
        D_sb = cpool.tile([128, 128], F32)
        nc.sync.dma_start(D_sb[:], d_D[:])
        Dr_sb = cpool.tile([128, 128], F32)
        nc.sync.dma_start(Dr_sb[:], d_Dr[:])
        ones_sb = cpool.tile([1, 128], F32)
        nc.sync.dma_start(ones_sb[:], d_ones[:])
        eps_sb = cpool.tile([1, 264], F32)
        nc.sync.dma_start(eps_sb[:], d_eps[:])
        pad_sb = cpool.tile([1, GROW], BF16)
        nc.sync.dma_start(pad_sb[:], d_pad[:])
        idx_sb = cpool.tile([128, meta["nidx"]], I16)
        nc.sync.dma_start(idx_sb[:], d_idx[:])

        if has_b:
            b_sb = cpool.tile([128, 3, 256], F32)
            nc.sync.dma_start(b_sb[:], d_b[:])
        er_res = cpool.tile([128, WPC, 4], F32)
        h_keep = cpool.tile([128, WPC, HID], BF16)

        kt_of_layer = [list(range(cfg.KT0)),
                       list(range(cfg.KT0, cfg.KT0 + cfg.KT)),
                       list(range(cfg.KT0 + cfg.KT, NK))]

        for l in range(3):
            # ---------------- projection phase ----------------
            for nt in range(WPC):
                kts = kt_of_layer[l]
                lhsTs = []
                if l == 0:
                    lhsTs.append(xT_sb[:, nt * 128:(nt + 1) * 128])
                else:
                    for ft in range(cfg.KT):
                        pst = ps_t.tile([128, 128], BF16, tag="pt")
                        nc.tensor.transpose(
                            pst[:],
                            h_keep[:, nt, ft * 128:(ft + 1) * 128],
                            id_sb[:])
                        hT = hpool.tile([128, 128], BF16, tag="lhsT")
                        nc.scalar.copy(hT[:], pst[:])
                        lhsTs.append(hT[:])
                pp = ps_p.tile([128, 264], F32, tag="pp")
                for k, (kt, lt) in enumerate(zip(kts, lhsTs)):
                    nc.tensor.matmul(pp[:], lt, w_sb[:, kt, :],
                                     start=(k == 0), stop=(k == len(kts) - 1))
                g_sb = gpool.tile([128, 264], BF16)
                nc.scalar.copy(g_sb[:, 0:256], pp[:, 0:256])
                nc.scalar.copy(g_sb[:, 256:264].bitcast(F32), pp[:, 256:260])
                nc.scalar.copy(er_res[:, nt, :], pp[:, 260:264])
                nc.sync.dma_start(g_loc[nt * 128:(nt + 1) * 128, 0:264],
                                  g_sb[:, 0:264])
                if l == 0:
                    pr = ps_p.tile([128, 256], F32, tag="pp")
                    nc.tensor.matmul(pr[:], lhsTs[0], rw_sb[:],
                                     start=True, stop=True)
                    # layer-0 residual (x @ resW0) parked in h_keep
                    nc.scalar.copy(h_keep[:, nt, :], pr[:])
            nc.sync.dma_start(g_loc[NPC:NPC + 1, :], pad_sb[:])

            g_full = g_fulls[l]
            nc.gpsimd.collective_compute(
                "AllGather", mybir.AluOpType.bypass,
                replica_groups=[list(range(num_cores))],
                ins=[g_loc.opt()], outs=[g_full.opt()])

            # ---------------- aggregation phase ----------------
            for w in range(WPC):
                Tw = int(T[w])
                lt_, ht_ = int(lo_t[w]), int(hi_t[w])
                t0 = int(toff[w])

                # er first-difference with bf16 hi/lo split
                er_dq = ps_q.tile([128, 4], F32, tag="edq")
                nc.tensor.matmul(er_dq[:], Dr_sb[:], er_res[:, w, :],
                                 start=True, stop=True)
                er8 = epool.tile([128, 8], BF16, tag="er8")
                nc.scalar.copy(er8[:, 0:4], er_dq[:])
                nc.vector.tensor_tensor(er8[:, 4:8], er_dq[:], er8[:, 0:4],
                                        mybir.AluOpType.subtract)

                fg = fgpool.tile([128, maxT, GROW], BF16)
                if lt_ > 0:
                    _gather(nc, fg[:, 0:lt_, :], g_full[0:cfg.LO_ROWS, :],
                            idx_sb, t0 * 8, lt_ * 128, GROW, q=w % 4)
                if ht_ > 0:
                    _gather(nc, fg[:, lt_:Tw, :],
                            g_full[cfg.HI_OFF:cfg.NPAD8, :], idx_sb,
                            (t0 + lt_) * 8, ht_ * 128, GROW, q=w % 4)

                # host-precomputed masks (layer-invariant):
                #  bm[d, t, e] = (e >= st[d, t])   for the er matmul
                #  bme[e, t, d] = (d <= dstrow)    for the aggregation matmul
                bm = bpool.tile([128, maxT, 128], F8, tag="bmd")
                nc.sync.dma_start(bm[:, 0:Tw, :], d_bm[:, t0:t0 + Tw, :])
                bme = bpool.tile([128, maxT, 128], F8, tag="bme")
                nc.sync.dma_start(bme[:, 0:Tw, :], d_bme[:, t0:t0 + Tw, :])

                # er per edge via telescoping (hi+lo columns)
                pe8 = ps_e.tile([128, maxT, 8], F32, tag="pe8")
                for t in range(Tw):
                    nc.tensor.matmul(pe8[:, t, :], bm[:, t, :], er8[:],
                                     start=True, stop=True)

                # logits + LeakyReLU + exp (exp lands in wsb cols 256:260)
                el_v = fg[:, 0:Tw, 256:264].bitcast(F32)    # [128, Tw, 4]
                e_sb = epool.tile([128, maxT, 4], F32, tag="e")
                nc.vector.tensor_tensor(e_sb[:, 0:Tw, :], el_v,
                                        pe8[:, 0:Tw, 0:4],
                                        mybir.AluOpType.add)
                nc.vector.tensor_tensor(e_sb[:, 0:Tw, :], e_sb[:, 0:Tw, :],
                                        pe8[:, 0:Tw, 4:8],
                                        mybir.AluOpType.add)
                nc.vector.scalar_tensor_tensor(
                    e_sb[:, 0:Tw, :], e_sb[:, 0:Tw, :], 0.2, e_sb[:, 0:Tw, :],
                    mybir.AluOpType.mult, mybir.AluOpType.max)
                exb = epool.tile([128, maxT, 4], BF16, tag="exb")
                nc.scalar.activation(exb[:, 0:Tw, :], e_sb[:, 0:Tw, :],
                                     AF.Exp)
                wsb = wpool.tile([128, maxT, 260], BF16)
                nc.vector.tensor_copy(wsb[:, 0:Tw, 256:260], exb[:, 0:Tw, :])

                # weighted feats (d-major): wsb[e,t,d,h] = fg[e,t,d,h]*ex[e,t,h]
                # inner stride-1 on h for both sources -> DVE 2x mode
                nc.vector.tensor_tensor(
                    wsb[:, 0:Tw, 0:256].rearrange("p t (d h) -> p t d h",
                                                  h=4),
                    fg[:, 0:Tw, 0:256].rearrange("p t (d h) -> p t d h", h=4),
                    exb[:, 0:Tw, :].unsqueeze(2).broadcast_to(
                        [128, Tw, 64, 4]),
                    mybir.AluOpType.mult)

                # suffix sums over tiles: pm[d] = sum_t BT_t.T @ wsb_t
                pm = ps_m.tile([128, 260], F32)
                for t in range(Tw):
                    nc.tensor.matmul(pm[:], bme[:, t, :], wsb[:, t, :],
                                     start=(t == 0), stop=(t == Tw - 1),
                                     skip_group_check=True)

                # telescope to per-dst segments + epsilon clamp
                s_sb = spool.tile([128, 260], F32)
                nc.scalar.copy(s_sb[:], pm[:])
                pm2 = ps_m2.tile([128, 260], F32)
                nc.tensor.matmul(pm2[:], D_sb[:], s_sb[:],
                                 start=True, stop=False)
                nc.tensor.matmul(pm2[:], ones_sb[:], eps_sb[0:1, 0:260],
                                 start=False, stop=True)

                rden = epool.tile([128, 4], F32, tag="rden")
                nc.vector.reciprocal(rden[:], pm2[:, 256:260])
                hn = hpool.tile([128, HID], F32, tag="hn")
                nc.vector.tensor_tensor(
                    hn[:].rearrange("p (d h) -> p d h", h=4),
                    pm2[:, 0:256].rearrange("p (d h) -> p d h", h=4),
                    rden[:].unsqueeze(1).broadcast_to([128, 64, 4]),
                    mybir.AluOpType.mult)
                nc.vector.tensor_tensor(hn[:], hn[:], h_keep[:, w, :],
                                        mybir.AluOpType.add)
                if has_b:
                    nc.vector.tensor_tensor(hn[:], hn[:], b_sb[:, l, :],
                                            mybir.AluOpType.add)
                if dbg_layer is not None and l == dbg_layer and w == 0:
                    d3 = hpool.tile([128, maxT * 128], F32, tag="d3sb")
                    nc.vector.tensor_copy(
                        d3[:, 0:maxT * 4].rearrange("p (t e) -> p t e",
                                                    t=maxT),
                        fg[:, 0:maxT, 256:264].bitcast(F32))
                    nc.vector.tensor_copy(
                        d3[:, maxT * 4:maxT * 4 + maxT * 64]
                        .rearrange("p (t e) -> p t e", t=maxT),
                        fg[:, 0:maxT, 0:64])
                    nc.sync.dma_start(d_dbg3[:], d3[:])
                    d4 = hpool.tile([128, maxT * 8], F32, tag="d4sb")
                    nc.vector.tensor_copy(
                        d4[:].rearrange("p (t e) -> p t e", t=maxT),
                        pe8[:, 0:maxT, :])
                    nc.sync.dma_start(d_dbg4[:], d4[:])
                if dbg_layer is not None and l == dbg_layer:
                    dsb = hpool.tile([128, HID], F32, tag="dsb")
                    nc.vector.tensor_copy(dsb[:], hn[:])
                    nc.sync.dma_start(d_dbg[w * 128:(w + 1) * 128, :], dsb[:])
                    d2 = hpool.tile([128, 524], F32, tag="d2sb")
                    nc.vector.tensor_copy(d2[:, 0:260], pm2[:, 0:260])
                    nc.vector.tensor_copy(d2[:, 260:264], er_res[:, w, :])
                    nc.vector.tensor_copy(d2[:, 264:524], s_sb[:])
                    nc.sync.dma_start(d_dbg2[w * 128:(w + 1) * 128, :],
                                      d2[:])
                if l < 2:
                    nc.scalar.activation(h_keep[:, w, :], hn[:], AF.Relu)
                else:
                    osb = hpool.tile([128, cfg.DH], F32, tag="osb")
                    nc.vector.tensor_reduce(
                        osb[:],
                        hn[:].rearrange("p (d h) -> p d h", h=4),
                        mybir.AxisListType.X, mybir.AluOpType.add)
                    nc.vector.tensor_scalar(osb[:], osb[:], 1.0 / cfg.H, None,
                                            mybir.AluOpType.mult)
                    nc.sync.dma_start(d_out[w * 128:(w + 1) * 128, :], osb[:])

    nc.compile()
    return nc


def make_in_maps(cfg, meta, wnp, num_cores):
    w_all, b_rep, rw, has_b = wnp
    D, Dr, ones, eps, padrow = _consts(cfg)
    ident = np.eye(128, dtype=np.float32)
    maps = []
    for c in range(num_cores):
        m = {
            "xT": _bf16(meta["xT"][c]),
            "w_all": _bf16(w_all), "resW0": _bf16(rw), "ident": _bf16(ident),
            "Dmat": D, "Drmat": Dr, "ones1": ones, "epsrow": eps,
            "padrow": padrow,
            "idx16": meta["idx16"][c],
            "bm": meta["bm"][c],
            "bme": meta["bme"][c],
        }
        if has_b:
            m["b_rep"] = b_rep.astype(np.float32)
        maps.append(m)
    return maps


def assemble_output(cfg, meta, results):
    out = np.empty((cfg.N, cfg.DH), np.float32)
    full = np.concatenate([r["out"] for r in results], axis=0)
    out[:] = full[meta["perm_pos"]]
    return out


def solve(x, edge_index, Ws, als, ars, bs, resW0, cfg, trace=False):
    meta = preprocess(x, edge_index, cfg)
    wnp = pack_weights(cfg, Ws, als, ars, bs, resW0)
    nc = build_program(cfg, meta, cfg.NCORES, has_b=wnp[3])
    in_maps = make_in_maps(cfg, meta, wnp, cfg.NCORES)
    res = bass_utils.run_bass_kernel_spmd(
        nc, in_maps, core_ids=list(range(cfg.NCORES)), trace=trace)
    out = assemble_output(cfg, meta, res.results)
    return out, res


def kernel(x, edge_index, W0, W1, W2, al0, al1, al2, ar0, ar1, ar2,
           b0, b1, b2, resW0):
    cfg = Cfg(n=50000, e=800000, fin=128, h=4, dh=64, ncores=8, wpc=49)
    out, _ = solve(np.asarray(x, np.float32), np.asarray(edge_index),
                   [np.asarray(W0, np.float32), np.asarray(W1, np.float32),
                    np.asarray(W2, np.float32)],
                   [np.asarray(al0, np.float32), np.asarray(al1, np.float32),
                    np.asarray(al2, np.float32)],
                   [np.asarray(ar0, np.float32), np.asarray(ar1, np.float32),
                    np.asarray(ar2, np.float32)],
                   [np.asarray(b0, np.float32), np.asarray(b1, np.float32),
                    np.asarray(b2, np.float32)],
                   np.asarray(resW0, np.float32), cfg)
    return out
